# revision 1
# baseline (speedup 1.0000x reference)
import sys
sys.path.insert(0, '/opt/trn_rl_repo')

"""Multi-head attention TP kernel for TRN2 — per-core program builder.

Sharding: 8 cores = 2 (batch) x 4 (head groups of 4 heads = 512 dims).
Each core computes, for its batch b and head-dim slice e:
    q = x[b] @ wq[e,:].T + bq[e]      (stored transposed: qT [E, S])
    k = x[b] @ wk[e,:].T + bk[e]      (kT [E, S])
    v = x[b] @ wv[e,:].T + bv[e]      (v [S, E])
    per head h (dh=128): ST = K-major score tiles, exp (no max-sub; scores
    bounded ~|3|), softmax denominator via DVE accumulate + all-ones
    broadcast matmul, AV accumulated unnormalized, normalized on eviction.
    partial_out = attn_out @ wo[:, e].T   ([S, D]; host sums 8 partials + bo)

Data path is bf16 (PE full rate, fast weight loads, LDWEIGHTS overlaps);
every accumulation (PSUM, softmax denominator) is fp32. x is shipped bf16
k-tile-major and stays resident in SBUF for both projection passes.
"""

import math

import numpy as np

import concourse.bass as bass
import concourse.tile as tile
from concourse import bacc, mybir

F32 = mybir.dt.float32
BF16 = mybir.dt.bfloat16
AF = mybir.ActivationFunctionType


def build_module(
    S=2048,          # sequence per core (one batch)
    D=2048,          # model dim
    E=512,           # head dims per core (4 heads x 128)
    bufs_es=3,
    enable_asserts=False,
):
    HD = 128
    SC = 512
    NK = D // HD        # proj contraction tiles
    NH = E // HD        # heads per core
    NSC = S // SC       # s-chunks / i-blocks
    NJ = S // HD        # attention j tiles
    ND = D // SC        # WO n-chunks
    NIT = S // HD       # WO i tiles
    scale = 1.0 / math.sqrt(HD)

    nc = bacc.Bacc(
        "TRN2",
        target_bir_lowering=False,
        debug=False,
        enable_asserts=enable_asserts,
        num_devices=8,
    )

    xr = nc.dram_tensor("xr", [HD, NK * S], BF16, kind="ExternalInput").ap()
    wqt = nc.dram_tensor("wqt", [HD, NK * E], BF16, kind="ExternalInput").ap()
    wkt = nc.dram_tensor("wkt", [HD, NK * E], BF16, kind="ExternalInput").ap()
    wvt = nc.dram_tensor("wvt", [HD, NK * E], BF16, kind="ExternalInput").ap()
    wot = nc.dram_tensor("wot", [HD, NH * D], BF16, kind="ExternalInput").ap()
    bqc = nc.dram_tensor("bqc", [HD, NH], F32, kind="ExternalInput").ap()
    bkc = nc.dram_tensor("bkc", [HD, NH], F32, kind="ExternalInput").ap()
    bvr = nc.dram_tensor("bvr", [1, E], BF16, kind="ExternalInput").ap()
    ones2d = nc.dram_tensor("ones2d", [HD, HD], BF16,
                            kind="ExternalInput").ap()
    out = nc.dram_tensor("out", [S, D], F32, kind="ExternalOutput").ap()

    with tile.TileContext(nc) as tc:
        with (
            tc.tile_pool(name="qkv", bufs=1) as qkv_pool,
            tc.tile_pool(name="consts", bufs=1) as consts,
        ):
            q_sb = qkv_pool.tile([HD, NH, S], BF16)
            k_sb = qkv_pool.tile([HD, NH, S], BF16)
            v_sb = qkv_pool.tile([HD, NJ, E], BF16)

            bq_sb = consts.tile([HD, NH], F32)
            bk_sb = consts.tile([HD, NH], F32)
            bv_sb = consts.tile([1, E], BF16)
            allones = consts.tile([HD, HD], BF16)    # bcast-sum stationary
            nc.sync.dma_start(out=bq_sb, in_=bqc)
            nc.sync.dma_start(out=bk_sb, in_=bkc)
            nc.sync.dma_start(out=bv_sb, in_=bvr)
            nc.sync.dma_start(out=allones, in_=ones2d)

            # -------- Phases A/B: Q,K then V projections, x resident -------
            with (
                tc.tile_pool(name="xall", bufs=1) as x_pool,
                tc.tile_pool(name="wqk", bufs=1) as w_pool,
            ):
                xall = x_pool.tile([HD, NK, S], BF16)
                xv = xall.rearrange("p k (t h) -> p k t h", h=HD)
                # split the x load across queues, 2 k-tiles per DMA
                for g in range(NK // 2):
                    nc.sync.dma_start(
                        out=xall[:, 2 * g:2 * g + 2, :],
                        in_=xr[:, 2 * g * S:(2 * g + 2) * S].rearrange(
                            "p (k s) -> p k s", s=S),
                    )
                wq_sb = w_pool.tile([HD, NK, E], BF16, tag="wq")
                wk_sb = w_pool.tile([HD, NK, E], BF16, tag="wk")
                for g4 in range(NK // 4):
                    nc.sync.dma_start(
                        out=wq_sb[:, 4 * g4:4 * g4 + 4, :],
                        in_=wqt[:, 4 * g4 * E:(4 * g4 + 4) * E].rearrange(
                            "p (k e) -> p k e", e=E))
                for g4 in range(NK // 4):
                    nc.sync.dma_start(
                        out=wk_sb[:, 4 * g4:4 * g4 + 4, :],
                        in_=wkt[:, 4 * g4 * E:(4 * g4 + 4) * E].rearrange(
                            "p (k e) -> p k e", e=E))

                with tc.tile_pool(name="psA", bufs=1, space="PSUM") as psA:
                    for si in range(NSC):
                        s0 = si * SC
                        psQ = [psA.tile([HD, SC], F32, tag=f"q{m}",
                                        name=f"psq{m}") for m in range(NH)]
                        psK = [psA.tile([HD, SC], F32, tag=f"k{m}",
                                        name=f"psk{m}") for m in range(NH)]
                        for kk in range(NK):
                            st = kk == 0
                            sp = kk == NK - 1
                            for m in range(NH):
                                nc.tensor.matmul(
                                    psQ[m],
                                    wq_sb[:, kk, m * HD:(m + 1) * HD],
                                    xall[:, kk, s0:s0 + SC],
                                    start=st, stop=sp,
                                )
                                nc.tensor.matmul(
                                    psK[m],
                                    wk_sb[:, kk, m * HD:(m + 1) * HD],
                                    xall[:, kk, s0:s0 + SC],
                                    start=st, stop=sp,
                                )
                        for m in range(NH):
                            nc.scalar.activation(
                                q_sb[:, m, s0:s0 + SC], psQ[m], AF.Identity,
                                bias=bq_sb[:, m:m + 1],
                            )
                            nc.scalar.activation(
                                k_sb[:, m, s0:s0 + SC], psK[m], AF.Identity,
                                bias=bk_sb[:, m:m + 1],
                            )

                # V projection reuses resident x as stationary tiles
                with tc.tile_pool(name="psB", bufs=1, space="PSUM") as psB:
                    wv_sb = w_pool.tile([HD, NK, E], BF16, tag="wq")
                    for g4 in range(NK // 4):
                        nc.sync.dma_start(
                            out=wv_sb[:, 4 * g4:4 * g4 + 4, :],
                            in_=wvt[:, 4 * g4 * E:(4 * g4 + 4) * E].rearrange(
                                "p (k e) -> p k e", e=E))
                    for si in range(NSC):
                        nmv = SC // HD
                        psV = [psB.tile([HD, E], F32, tag=f"v{m}",
                                        name=f"psv{m}") for m in range(nmv)]
                        for kk in range(NK):
                            for mv in range(nmv):
                                nc.tensor.matmul(
                                    psV[mv],
                                    xv[:, kk, si * nmv + mv, :],
                                    wv_sb[:, kk, :],
                                    start=(kk == 0), stop=False,
                                )
                        for mv in range(nmv):
                            # bias: ones^T @ bv_row as a final K=1 matmul
                            nc.tensor.matmul(
                                psV[mv], allones[0:1, :], bv_sb,
                                start=False, stop=True,
                            )
                            nc.scalar.copy(v_sb[:, si * nmv + mv, :], psV[mv])

            # ---------------- Phase C: attention ----------------
            with tc.tile_pool(name="outT", bufs=1) as outT_pool:
                outT_sb = outT_pool.tile([HD, NH, S], BF16)
                with (
                    tc.tile_pool(name="attws", bufs=2) as ws_pool,
                    tc.tile_pool(name="es", bufs=bufs_es) as es_pool,
                    tc.tile_pool(name="psS", bufs=2, space="PSUM") as psS_pool,
                    tc.tile_pool(name="psO", bufs=2, space="PSUM") as psO_pool,
                    tc.tile_pool(name="psN", bufs=2, space="PSUM") as psN_pool,
                ):
                    for h in range(NH):
                        for ib in range(NSC):
                            i0 = ib * SC
                            psO = psO_pool.tile([HD, SC], F32, tag="o")
                            ps_bc = psN_pool.tile([HD, SC], F32, tag="bc")
                            es_t = [None] * NJ

                            def emit_av(j, h=h, psO=psO, ps_bc=ps_bc,
                                        es_t=es_t):
                                nc.tensor.matmul(
                                    psO, v_sb[:, j, h * HD:(h + 1) * HD],
                                    es_t[j],
                                    start=(j == 0), stop=(j == NJ - 1),
                                )
                                nc.tensor.matmul(
                                    ps_bc, allones, es_t[j],
                                    start=(j == 0), stop=(j == NJ - 1),
                                )

                            for j in range(NJ):
                                psS = psS_pool.tile([HD, SC], F32, tag="s")
                                nc.tensor.matmul(
                                    psS, k_sb[:, h, j * HD:(j + 1) * HD],
                                    q_sb[:, h, i0:i0 + SC],
                                    start=True, stop=True,
                                )
                                if j >= 1:
                                    emit_av(j - 1)
                                es = es_pool.tile([HD, SC], BF16, tag="es",
                                                  name="es")
                                nc.scalar.activation(es, psS, AF.Exp,
                                                     scale=scale)
                                es_t[j] = es
                            emit_av(NJ - 1)
                            recip = ws_pool.tile([HD, SC], F32, tag="recip")
                            nc.vector.reciprocal(recip, ps_bc)
                            nc.vector.tensor_mul(
                                outT_sb[:, h, i0:i0 + SC], psO, recip)

                # ---------------- Phase D: WO projection ----------------
                with (
                    tc.tile_pool(name="wo", bufs=1) as wo_pool,
                    tc.tile_pool(name="og", bufs=2) as og_pool,
                    tc.tile_pool(name="psW", bufs=4, space="PSUM") as psW_pool,
                ):
                    wo_sb = wo_pool.tile([HD, NH, D], BF16)
                    nc.sync.dma_start(
                        out=wo_sb, in_=wot.rearrange("p (k d) -> p k d", d=D))
                    for it in range(NIT):
                        og = og_pool.tile([HD, D], F32, tag="og")
                        for nn in range(ND):
                            psW = psW_pool.tile([HD, SC], F32, tag="w")
                            for kk in range(NH):
                                nc.tensor.matmul(
                                    psW,
                                    outT_sb[:, kk, it * HD:(it + 1) * HD],
                                    wo_sb[:, kk, nn * SC:(nn + 1) * SC],
                                    start=(kk == 0), stop=(kk == NH - 1),
                                )
                            if nn % 2 == 0:
                                nc.scalar.copy(
                                    og[:, nn * SC:(nn + 1) * SC], psW)
                            else:
                                nc.vector.tensor_copy(
                                    og[:, nn * SC:(nn + 1) * SC], psW)
                        nc.sync.dma_start(
                            out=out[it * HD:(it + 1) * HD, :], in_=og)

    nc.compile()
    return nc


# ---------------------------------------------------------------------------
# Host-side sharding helpers
# ---------------------------------------------------------------------------

def _bf16(a):
    import ml_dtypes
    return np.asarray(a).astype(ml_dtypes.bfloat16)


def make_in_map(x_b, wq_e, bq_e, wk_e, bk_e, wv_e, bv_e, wo_e):
    """Per-core input dict. x_b [S, D]; w*_e [E, D] row slices; wo_e [D, E]
    column slice; b*_e [E]."""
    E = wq_e.shape[0]
    S, D = x_b.shape
    HD = 128
    NH = E // HD
    NK = D // HD

    def wrelayout(wT):  # [D, E'] -> [HD, NK*E'] with k-tile-major columns
        Ew = wT.shape[1]
        return _bf16(
            wT.reshape(NK, HD, Ew).transpose(1, 0, 2).reshape(HD, NK * Ew))

    xT = x_b.T  # [D, S]
    return {
        "xr": _bf16(xT.reshape(NK, HD, S).transpose(1, 0, 2)
                    .reshape(HD, NK * S)),
        "wqt": wrelayout(wq_e.T),
        "wkt": wrelayout(wk_e.T),
        "wvt": wrelayout(wv_e.T),
        "wot": _bf16(
            wo_e.T.reshape(NH, HD, D).transpose(1, 0, 2).reshape(HD, NH * D)),
        "bqc": np.ascontiguousarray(bq_e.reshape(NH, HD).T),
        "bkc": np.ascontiguousarray(bk_e.reshape(NH, HD).T),
        "bvr": _bf16(bv_e.reshape(1, E)),
        "ones2d": _bf16(np.ones((HD, HD), np.float32)),
    }


def core_reference(x_b, wq_e, bq_e, wk_e, bk_e, wv_e, bv_e, wo_e):
    """Numpy reference for one core's partial output."""
    HD = 128
    q = x_b @ wq_e.T + bq_e
    k = x_b @ wk_e.T + bk_e
    v = x_b @ wv_e.T + bv_e
    E = q.shape[1]
    outs = []
    for h in range(E // HD):
        qh = q[:, h * HD:(h + 1) * HD]
        kh = k[:, h * HD:(h + 1) * HD]
        vh = v[:, h * HD:(h + 1) * HD]
        s = (qh @ kh.T) / math.sqrt(HD)
        p = np.exp(s)
        outs.append((p @ vh) / p.sum(-1, keepdims=True))
    o = np.concatenate(outs, axis=1)
    return o @ wo_e.T


# ---------------------------------------------------------------------------
# Entry point: full-input kernel with internal 8-way sharding
# ---------------------------------------------------------------------------

import os as _os

_NC_CACHE = {}


def _get_module():
    if "nc" not in _NC_CACHE:
        _NC_CACHE["nc"] = build_module(S=2048, D=2048, E=512)
    return _NC_CACHE["nc"]


def kernel(x, wq, bq, wk, bk, wv, bv, wo, bo):
    """Full inputs -> full output. 8 cores = 2 (batch) x 4 (head-group)."""
    from concourse import bass_utils

    x = np.asarray(x, dtype=np.float32)
    wq, bq = np.asarray(wq, np.float32), np.asarray(bq, np.float32)
    wk, bk = np.asarray(wk, np.float32), np.asarray(bk, np.float32)
    wv, bv = np.asarray(wv, np.float32), np.asarray(bv, np.float32)
    wo, bo = np.asarray(wo, np.float32), np.asarray(bo, np.float32)

    E = 512
    nc = _get_module()
    in_maps = []
    for c in range(8):
        b, g = divmod(c, 4)
        e = slice(g * E, (g + 1) * E)
        in_maps.append(make_in_map(
            x[b], wq[e], bq[e], wk[e], bk[e], wv[e], bv[e], wo[:, e]))

    trace = bool(int(_os.environ.get("ATTN_TRACE", "0")))
    kw = {}
    if trace:
        tmpdir = _os.environ.get("ATTN_TRACE_DIR") or None
        kw = dict(trace=True, tmpdir=tmpdir, trace_cores=[0])
    res = bass_utils.run_bass_kernel_spmd(
        nc, in_maps, core_ids=list(range(8)), **kw)
    if trace:
        print(f"HW exec time: {res.exec_time_ns} ns")
        _NC_CACHE["last_results"] = res

    y = np.empty((2, 2048, 2048), np.float32)
    for b in range(2):
        acc = res.results[4 * b]["out"].copy()
        for g in range(1, 4):
            acc += res.results[4 * b + g]["out"]
        y[b] = acc + bo
    return y



# revision 3
# speedup vs baseline: 1.1395x; 1.1395x over previous
import sys
sys.path.insert(0, '/opt/trn_rl_repo')

"""Multi-head attention TP kernel for TRN2 — per-core program builder.

Sharding: 8 cores = 2 (batch) x 4 (head groups of 4 heads = 512 dims).
Each core computes, for its batch b and head-dim slice e:
    q = x[b] @ wq[e,:].T + bq[e]      (stored transposed: qT [E, S])
    k = x[b] @ wk[e,:].T + bk[e]      (kT [E, S])
    v = x[b] @ wv[e,:].T + bv[e]      (v [S, E])
    per head h (dh=128): score tiles K-major, exp (no max-sub; scores
    bounded ~|3|), softmax denominator via all-ones broadcast matmul,
    AV accumulated unnormalized, normalized on eviction via
    rsqrt(denom) applied twice (avoids slow DVE reciprocal).
    partial_out = attn_out @ wo[:, e].T   ([S, D] bf16; host sums 8
    partials + bo in fp32)

v2 notes (vs baseline):
- DMA order: HWDGE ring is FIFO per issuing engine, so the first-needed
  weights go first (wq h0/h1 -> x -> wq h2/h3 -> wk -> wv; wo at phase-C
  start). Baseline queued all of x first and idled the PE ~39us.
- Phase A as separate Q/K passes, h-outer, 4-bank PSUM quads ping-ponged
  across h; stationary weight tile reused by 4 consecutive matmuls; one
  2048-wide activation per head amortizes ACT fixed cost.
- Phase C processes j in pairs: 2-bank psS tile, one 1024-wide exp.
- Normalization: rsqrt on ACT + two DVE multiplies.
- Output bf16 (halves output DMA).
"""

import math

import numpy as np

import concourse.bass as bass
import concourse.tile as tile
from concourse import bacc, mybir

F32 = mybir.dt.float32
BF16 = mybir.dt.bfloat16
AF = mybir.ActivationFunctionType


def build_module(
    S=2048,          # sequence per core (one batch)
    D=2048,          # model dim
    E=512,           # head dims per core (4 heads x 128)
    enable_asserts=False,
):
    HD = 128
    SC = 512
    NK = D // HD        # proj contraction tiles
    NH = E // HD        # heads per core
    NSC = S // SC       # s-chunks / i-blocks
    NJ = S // HD        # attention j tiles
    NP = NJ // 2        # attention j pairs
    ND = D // SC        # WO n-chunks
    NIT = S // HD       # WO i tiles
    scale = 1.0 / math.sqrt(HD)

    nc = bacc.Bacc(
        "TRN2",
        target_bir_lowering=False,
        debug=False,
        enable_asserts=enable_asserts,
        num_devices=8,
    )

    # host-side layouts (see make_in_map):
    #   xr  [HD, NK*S]       k-tile-major xT
    #   wqt [HD, NH*NK*HD]   h-major, then k-tile, then head-col
    xr = nc.dram_tensor("xr", [HD, NK * S], BF16, kind="ExternalInput").ap()
    wqt = nc.dram_tensor("wqt", [HD, NH * NK * HD], BF16,
                         kind="ExternalInput").ap()
    wkt = nc.dram_tensor("wkt", [HD, NH * NK * HD], BF16,
                         kind="ExternalInput").ap()
    wvt = nc.dram_tensor("wvt", [HD, NK * E], BF16, kind="ExternalInput").ap()
    wot = nc.dram_tensor("wot", [HD, NH * D], BF16, kind="ExternalInput").ap()
    bqc = nc.dram_tensor("bqc", [HD, NH], F32, kind="ExternalInput").ap()
    bkc = nc.dram_tensor("bkc", [HD, NH], F32, kind="ExternalInput").ap()
    bvr = nc.dram_tensor("bvr", [1, E], BF16, kind="ExternalInput").ap()
    ones2d = nc.dram_tensor("ones2d", [HD, HD], BF16,
                            kind="ExternalInput").ap()
    out = nc.dram_tensor("out", [S, D], BF16, kind="ExternalOutput").ap()

    with tile.TileContext(nc) as tc:
        with (
            tc.tile_pool(name="qkv", bufs=1) as qkv_pool,
            tc.tile_pool(name="consts", bufs=1) as consts,
        ):
            q_sb = qkv_pool.tile([HD, NH, S], BF16)
            k_sb = qkv_pool.tile([HD, NH, S], BF16)
            v_sb = qkv_pool.tile([HD, NJ, E], BF16)

            bq_sb = consts.tile([HD, NH], F32)
            bk_sb = consts.tile([HD, NH], F32)
            bv_sb = consts.tile([1, E], BF16)
            allones = consts.tile([HD, HD], BF16)    # bcast-sum stationary
            nc.sync.dma_start(out=bq_sb, in_=bqc)
            nc.sync.dma_start(out=bk_sb, in_=bkc)
            nc.sync.dma_start(out=bv_sb, in_=bvr)
            nc.sync.dma_start(out=allones, in_=ones2d)

            # -------- Phases A/B: Q,K then V projections, x resident -------
            with (
                tc.tile_pool(name="xall", bufs=1) as x_pool,
                tc.tile_pool(name="wqk", bufs=1) as w_pool,
            ):
                xall = x_pool.tile([HD, NK, S], BF16)
                xv = xall.rearrange("p k (t h) -> p k t h", h=HD)
                wq_sb = w_pool.tile([HD, NH, NK, HD], BF16, tag="wq")
                wk_sb = w_pool.tile([HD, NH, NK, HD], BF16, tag="wk")
                wv_sb = w_pool.tile([HD, NK, E], BF16, tag="wv")

                wqr = wqt.rearrange("p (h k c) -> p h k c", h=NH, k=NK)
                wkr = wkt.rearrange("p (h k c) -> p h k c", h=NH, k=NK)

                # DMA issue order == arrival order (FIFO ring):
                # wq h0/h1 first so the very first matmuls can start, then
                # x (k-tile-major, consumed k-outer), then the rest.
                nc.sync.dma_start(out=wq_sb[:, 0:2], in_=wqr[:, 0:2])
                for g in range(NK // 2):
                    nc.sync.dma_start(
                        out=xall[:, 2 * g:2 * g + 2, :],
                        in_=xr[:, 2 * g * S:(2 * g + 2) * S].rearrange(
                            "p (k s) -> p k s", s=S),
                    )
                nc.sync.dma_start(out=wq_sb[:, 2:4], in_=wqr[:, 2:4])
                nc.sync.dma_start(out=wk_sb[:, 0:2], in_=wkr[:, 0:2])
                nc.sync.dma_start(out=wk_sb[:, 2:4], in_=wkr[:, 2:4])
                for g4 in range(NK // 4):
                    nc.sync.dma_start(
                        out=wv_sb[:, 4 * g4:4 * g4 + 4, :],
                        in_=wvt[:, 4 * g4 * E:(4 * g4 + 4) * E].rearrange(
                            "p (k e) -> p k e", e=E))

                # ---- Q pass, then K pass: h-outer, stationary reused x4 ---
                with tc.tile_pool(name="psA", bufs=2, space="PSUM") as psA:
                    for (w_sb, b_sb, dst) in ((wq_sb, bq_sb, q_sb),
                                              (wk_sb, bk_sb, k_sb)):
                        for h in range(NH):
                            quad = psA.tile([HD, NSC, SC], F32, tag="q")
                            for kk in range(NK):
                                for si in range(NSC):
                                    nc.tensor.matmul(
                                        quad[:, si, :],
                                        w_sb[:, h, kk, :],
                                        xall[:, kk, si * SC:(si + 1) * SC],
                                        start=(kk == 0), stop=(kk == NK - 1),
                                    )
                            nc.scalar.activation(
                                dst[:, h, :], quad, AF.Identity,
                                bias=b_sb[:, h:h + 1],
                            )

                # ---- V projection reuses resident x as stationary tiles ---
                with tc.tile_pool(name="psB", bufs=2, space="PSUM") as psB:
                    for si in range(NSC):
                        nmv = SC // HD
                        psV = psB.tile([HD, nmv, E], F32, tag="v")
                        for kk in range(NK):
                            for mv in range(nmv):
                                nc.tensor.matmul(
                                    psV[:, mv, :],
                                    xv[:, kk, si * nmv + mv, :],
                                    wv_sb[:, kk, :],
                                    start=(kk == 0), stop=False,
                                )
                        for mv in range(nmv):
                            # bias: ones^T @ bv_row as a final K=1 matmul
                            nc.tensor.matmul(
                                psV[:, mv, :], allones[0:1, :], bv_sb,
                                start=False, stop=True,
                            )
                        nc.scalar.activation(
                            v_sb[:, si * nmv:(si + 1) * nmv, :], psV,
                            AF.Identity)

            # ---------------- Phase C: attention ----------------
            with tc.tile_pool(name="outT", bufs=1) as outT_pool:
                outT_sb = outT_pool.tile([HD, NH, S], BF16)
                wo_sb = outT_pool.tile([HD, NH, D], BF16)
                # prefetch WO now; x/w pools above are closed
                nc.sync.dma_start(
                    out=wo_sb, in_=wot.rearrange("p (k d) -> p k d", d=D))
                with (
                    tc.tile_pool(name="attws", bufs=2) as ws_pool,
                    tc.tile_pool(name="es", bufs=4) as es_pool,
                    tc.tile_pool(name="psS", bufs=2, space="PSUM") as psS_pool,
                    tc.tile_pool(name="psO", bufs=2, space="PSUM") as psO_pool,
                    tc.tile_pool(name="psN", bufs=2, space="PSUM") as psN_pool,
                ):
                    for h in range(NH):
                        for ib in range(NSC):
                            i0 = ib * SC
                            psO = psO_pool.tile([HD, SC], F32, tag="o")
                            ps_bc = psN_pool.tile([HD, SC], F32, tag="bc")
                            es_t = [None] * NP

                            def emit_av(p, h=h, psO=psO, ps_bc=ps_bc,
                                        es_t=es_t):
                                for jj in range(2):
                                    j = 2 * p + jj
                                    st = j == 0
                                    sp = j == NJ - 1
                                    nc.tensor.matmul(
                                        psO,
                                        v_sb[:, j, h * HD:(h + 1) * HD],
                                        es_t[p][:, jj, :],
                                        start=st, stop=sp,
                                    )
                                    nc.tensor.matmul(
                                        ps_bc, allones, es_t[p][:, jj, :],
                                        start=st, stop=sp,
                                    )

                            for p in range(NP):
                                psS = psS_pool.tile([HD, 2, SC], F32, tag="s")
                                for jj in range(2):
                                    j = 2 * p + jj
                                    nc.tensor.matmul(
                                        psS[:, jj, :],
                                        k_sb[:, h, j * HD:(j + 1) * HD],
                                        q_sb[:, h, i0:i0 + SC],
                                        start=True, stop=True,
                                    )
                                if p >= 2:
                                    emit_av(p - 2)
                                es = es_pool.tile([HD, 2, SC], BF16, tag="es",
                                                  name="es")
                                nc.scalar.activation(es, psS, AF.Exp,
                                                     scale=scale)
                                es_t[p] = es
                            emit_av(NP - 2)
                            emit_av(NP - 1)
                            # normalize: approx 1/d (18-bit, ample for bf16)
                            rcp = ws_pool.tile([HD, SC], F32, tag="rcp")
                            nc.vector.reciprocal_approx_fast(
                                out=rcp, in_=ps_bc)
                            nc.vector.tensor_mul(
                                outT_sb[:, h, i0:i0 + SC], psO, rcp)

                # ---------------- Phase D: WO projection ----------------
                with (
                    tc.tile_pool(name="og", bufs=2) as og_pool,
                    tc.tile_pool(name="psW", bufs=2, space="PSUM") as psW_pool,
                ):
                    for it in range(NIT):
                        og = og_pool.tile([HD, D], BF16, tag="og")
                        psW = psW_pool.tile([HD, ND, SC], F32, tag="w")
                        for kk in range(NH):
                            for nn in range(ND):
                                nc.tensor.matmul(
                                    psW[:, nn, :],
                                    outT_sb[:, kk, it * HD:(it + 1) * HD],
                                    wo_sb[:, kk, nn * SC:(nn + 1) * SC],
                                    start=(kk == 0), stop=(kk == NH - 1),
                                )
                        nc.scalar.activation(og, psW, AF.Identity)
                        nc.sync.dma_start(
                            out=out[it * HD:(it + 1) * HD, :], in_=og)

    nc.compile()
    return nc


# ---------------------------------------------------------------------------
# Host-side sharding helpers
# ---------------------------------------------------------------------------

def _bf16(a):
    import ml_dtypes
    return np.asarray(a).astype(ml_dtypes.bfloat16)


def make_in_map(x_b, wq_e, bq_e, wk_e, bk_e, wv_e, bv_e, wo_e):
    """Per-core input dict. x_b [S, D]; w*_e [E, D] row slices; wo_e [D, E]
    column slice; b*_e [E]."""
    E = wq_e.shape[0]
    S, D = x_b.shape
    HD = 128
    NH = E // HD
    NK = D // HD

    def w_hmajor(w_e):  # [E, D] -> [HD, NH*NK*HD]: p=k-row, (h, k, col)
        # entry [p, h, k, c] = w_e[h*HD+c, k*HD+p]
        t = w_e.reshape(NH, HD, NK, HD)        # [h, c, k, p]
        t = t.transpose(3, 0, 2, 1)            # [p, h, k, c]
        return _bf16(t.reshape(HD, NH * NK * HD))

    def wrelayout(wT):  # [D, E'] -> [HD, NK*E'] with k-tile-major columns
        Ew = wT.shape[1]
        return _bf16(
            wT.reshape(NK, HD, Ew).transpose(1, 0, 2).reshape(HD, NK * Ew))

    xT = x_b.T  # [D, S]
    return {
        "xr": _bf16(xT.reshape(NK, HD, S).transpose(1, 0, 2)
                    .reshape(HD, NK * S)),
        "wqt": w_hmajor(wq_e),
        "wkt": w_hmajor(wk_e),
        "wvt": wrelayout(wv_e.T),
        "wot": _bf16(
            wo_e.T.reshape(NH, HD, D).transpose(1, 0, 2).reshape(HD, NH * D)),
        "bqc": np.ascontiguousarray(bq_e.reshape(NH, HD).T),
        "bkc": np.ascontiguousarray(bk_e.reshape(NH, HD).T),
        "bvr": _bf16(bv_e.reshape(1, E)),
        "ones2d": _bf16(np.ones((HD, HD), np.float32)),
    }


def core_reference(x_b, wq_e, bq_e, wk_e, bk_e, wv_e, bv_e, wo_e):
    """Numpy reference for one core's partial output."""
    HD = 128
    q = x_b @ wq_e.T + bq_e
    k = x_b @ wk_e.T + bk_e
    v = x_b @ wv_e.T + bv_e
    E = q.shape[1]
    outs = []
    for h in range(E // HD):
        qh = q[:, h * HD:(h + 1) * HD]
        kh = k[:, h * HD:(h + 1) * HD]
        vh = v[:, h * HD:(h + 1) * HD]
        s = (qh @ kh.T) / math.sqrt(HD)
        p = np.exp(s)
        outs.append((p @ vh) / p.sum(-1, keepdims=True))
    o = np.concatenate(outs, axis=1)
    return o @ wo_e.T


# ---------------------------------------------------------------------------
# Entry point: full-input kernel with internal 8-way sharding
# ---------------------------------------------------------------------------

import os as _os

_NC_CACHE = {}


def _get_module():
    if "nc" not in _NC_CACHE:
        _NC_CACHE["nc"] = build_module(S=2048, D=2048, E=512)
    return _NC_CACHE["nc"]


def kernel(x, wq, bq, wk, bk, wv, bv, wo, bo):
    """Full inputs -> full output. 8 cores = 2 (batch) x 4 (head-group)."""
    from concourse import bass_utils

    x = np.asarray(x, dtype=np.float32)
    wq, bq = np.asarray(wq, np.float32), np.asarray(bq, np.float32)
    wk, bk = np.asarray(wk, np.float32), np.asarray(bk, np.float32)
    wv, bv = np.asarray(wv, np.float32), np.asarray(bv, np.float32)
    wo, bo = np.asarray(wo, np.float32), np.asarray(bo, np.float32)

    E = 512
    nc = _get_module()
    in_maps = []
    for c in range(8):
        b, g = divmod(c, 4)
        e = slice(g * E, (g + 1) * E)
        in_maps.append(make_in_map(
            x[b], wq[e], bq[e], wk[e], bk[e], wv[e], bv[e], wo[:, e]))

    trace = bool(int(_os.environ.get("ATTN_TRACE", "0")))
    kw = {}
    if trace:
        tmpdir = _os.environ.get("ATTN_TRACE_DIR") or None
        kw = dict(trace=True, tmpdir=tmpdir, trace_cores=[0])
    res = bass_utils.run_bass_kernel_spmd(
        nc, in_maps, core_ids=list(range(8)), **kw)
    if trace:
        print(f"HW exec time: {res.exec_time_ns} ns")
        _NC_CACHE["last_results"] = res

    y = np.empty((2, 2048, 2048), np.float32)
    for b in range(2):
        acc = res.results[4 * b]["out"].astype(np.float32)
        for g in range(1, 4):
            acc += res.results[4 * b + g]["out"].astype(np.float32)
        y[b] = acc + bo
    return y


# revision 7
# speedup vs baseline: 1.4823x; 1.3008x over previous
import sys
sys.path.insert(0, '/opt/trn_rl_repo')

"""Multi-head attention TP kernel for TRN2 — per-core program builder.

Sharding: 8 cores = 2 (batch) x 4 (head groups of 4 heads = 512 dims).
Each core computes, for its batch b and head-dim slice e:
    q = x[b] @ wq[e,:].T + bq[e]      (stored transposed: qT [E, S])
    k = x[b] @ wk[e,:].T + bk[e]      (kT [E, S])
    v = x[b] @ wv[e,:].T + bv[e]      (v [S, E])
    per head h (dh=128): score tiles K-major, exp (no max-sub; scores
    bounded ~|3|), softmax denominator via all-ones broadcast matmul,
    AV accumulated unnormalized, normalized on eviction via
    rsqrt(denom) applied twice (avoids slow DVE reciprocal).
    partial_out = attn_out @ wo[:, e].T   ([S, D] bf16; host sums 8
    partials + bo in fp32)

v2 notes (vs baseline):
- DMA order: HWDGE ring is FIFO per issuing engine, so the first-needed
  weights go first (wq h0/h1 -> x -> wq h2/h3 -> wk -> wv; wo at phase-C
  start). Baseline queued all of x first and idled the PE ~39us.
- Phase A as separate Q/K passes, h-outer, 4-bank PSUM quads ping-ponged
  across h; stationary weight tile reused by 4 consecutive matmuls; one
  2048-wide activation per head amortizes ACT fixed cost.
- Phase C processes j in pairs: 2-bank psS tile, one 1024-wide exp.
- Normalization: rsqrt on ACT + two DVE multiplies.
- Output bf16 (halves output DMA).
"""

import math

import numpy as np

import concourse.bass as bass
import concourse.tile as tile
from concourse import bacc, mybir

F32 = mybir.dt.float32
BF16 = mybir.dt.bfloat16
AF = mybir.ActivationFunctionType


def build_module(
    S=2048,          # sequence per core (one batch)
    D=2048,          # model dim
    E=512,           # head dims per core (4 heads x 128)
    enable_asserts=False,
):
    HD = 128
    SC = 512
    NK = D // HD        # proj contraction tiles
    NH = E // HD        # heads per core
    NSC = S // SC       # s-chunks / i-blocks
    NJ = S // HD        # attention j tiles
    NP = NJ // 2        # attention j pairs
    ND = D // SC        # WO n-chunks
    NIT = S // HD       # WO i tiles
    scale = 1.0 / math.sqrt(HD)

    nc = bacc.Bacc(
        "TRN2",
        target_bir_lowering=False,
        debug=False,
        enable_asserts=enable_asserts,
        num_devices=8,
    )

    # host-side layouts (see make_in_map):
    #   xr  [HD, NK*S]       k-tile-major xT
    #   wqt [HD, NH*NK*HD]   h-major, then k-tile, then head-col
    xr = nc.dram_tensor("xr", [HD, NK * S], BF16, kind="ExternalInput").ap()
    wqt = nc.dram_tensor("wqt", [HD, NH * NK * HD], BF16,
                         kind="ExternalInput").ap()
    wkt = nc.dram_tensor("wkt", [HD, NH * NK * HD], BF16,
                         kind="ExternalInput").ap()
    wvt = nc.dram_tensor("wvt", [HD, NK * E], BF16, kind="ExternalInput").ap()
    wot = nc.dram_tensor("wot", [HD, NH * D], BF16, kind="ExternalInput").ap()
    bqc = nc.dram_tensor("bqc", [HD, NH], F32, kind="ExternalInput").ap()
    bkc = nc.dram_tensor("bkc", [HD, NH], F32, kind="ExternalInput").ap()
    bvr = nc.dram_tensor("bvr", [1, E], BF16, kind="ExternalInput").ap()
    ones2d = nc.dram_tensor("ones2d", [HD, HD], BF16,
                            kind="ExternalInput").ap()
    out = nc.dram_tensor("out", [S, D], BF16, kind="ExternalOutput").ap()

    with tile.TileContext(nc) as tc:
        with (
            tc.tile_pool(name="qkv", bufs=1) as qkv_pool,
            tc.tile_pool(name="consts", bufs=1) as consts,
        ):
            q_sb = qkv_pool.tile([HD, NH, S], BF16)
            k_sb = qkv_pool.tile([HD, NH, S], BF16)
            v_sb = qkv_pool.tile([HD, NJ, E], BF16)

            bq_sb = consts.tile([HD, NH], F32)
            bk_sb = consts.tile([HD, NH], F32)
            bv_sb = consts.tile([1, E], BF16)
            allones = consts.tile([HD, HD], BF16)    # bcast-sum stationary
            nc.sync.dma_start(out=bq_sb, in_=bqc)
            nc.sync.dma_start(out=bk_sb, in_=bkc)
            nc.sync.dma_start(out=bv_sb, in_=bvr)
            nc.sync.dma_start(out=allones, in_=ones2d)

            # -------- Phases A/B: Q,K then V projections, x resident -------
            with (
                tc.tile_pool(name="xall", bufs=1) as x_pool,
                tc.tile_pool(name="wqk", bufs=1) as w_pool,
            ):
                xall = x_pool.tile([HD, NK, S], BF16)
                xv = xall.rearrange("p k (t h) -> p k t h", h=HD)
                wq_sb = w_pool.tile([HD, NH, NK, HD], BF16, tag="wq")
                wk_sb = w_pool.tile([HD, NH, NK, HD], BF16, tag="wk")
                wv_sb = w_pool.tile([HD, NK, E], BF16, tag="wv")

                wqr = wqt.rearrange("p (h k c) -> p h k c", h=NH, k=NK)
                wkr = wkt.rearrange("p (h k c) -> p h k c", h=NH, k=NK)

                # DMA issue order == arrival order (FIFO ring):
                # wq h0/h1 first so the very first matmuls can start, then
                # x (k-tile-major, consumed k-outer), then the rest.
                nc.sync.dma_start(out=wq_sb[:, 0:1], in_=wqr[:, 0:1])
                nc.sync.dma_start(out=wq_sb[:, 1:2], in_=wqr[:, 1:2])
                for g in range(NK // 2):
                    nc.sync.dma_start(
                        out=xall[:, 2 * g:2 * g + 2, :],
                        in_=xr[:, 2 * g * S:(2 * g + 2) * S].rearrange(
                            "p (k s) -> p k s", s=S),
                    )
                nc.sync.dma_start(out=wq_sb[:, 2:4], in_=wqr[:, 2:4])
                nc.sync.dma_start(out=wk_sb[:, 0:2], in_=wkr[:, 0:2])
                nc.sync.dma_start(out=wk_sb[:, 2:4], in_=wkr[:, 2:4])
                for g4 in range(NK // 4):
                    nc.sync.dma_start(
                        out=wv_sb[:, 4 * g4:4 * g4 + 4, :],
                        in_=wvt[:, 4 * g4 * E:(4 * g4 + 4) * E].rearrange(
                            "p (k e) -> p k e", e=E))

                # warm-up matmuls on the (tiny, early-arriving) ones tile:
                # keeps the PE HAM counter busy during the input DMA wait
                # so real matmuls start at full clock
                with tc.tile_pool(name="psWm", bufs=1, space="PSUM") as psWm:
                    warm = psWm.tile([HD, HD], F32)
                    for _ in range(24):
                        nc.tensor.matmul(warm, allones, allones,
                                         start=True, stop=True)

                # ---- Q pass, then K pass: h-outer, stationary reused x4 ---
                with tc.tile_pool(name="psA", bufs=2, space="PSUM") as psA:
                    for (w_sb, b_sb, dst) in ((wq_sb, bq_sb, q_sb),
                                              (wk_sb, bk_sb, k_sb)):
                        for h in range(NH):
                            quad = psA.tile([HD, NSC, SC], F32, tag="q")
                            for kk in range(NK):
                                for si in range(NSC):
                                    nc.tensor.matmul(
                                        quad[:, si, :],
                                        w_sb[:, h, kk, :],
                                        xall[:, kk, si * SC:(si + 1) * SC],
                                        start=(kk == 0), stop=(kk == NK - 1),
                                    )
                            nc.scalar.activation(
                                dst[:, h, :], quad, AF.Identity,
                                bias=b_sb[:, h:h + 1],
                            )

                # ---- V projection reuses resident x as stationary tiles ---
                with tc.tile_pool(name="psB", bufs=2, space="PSUM") as psB:
                    for si in range(NSC):
                        nmv = SC // HD
                        psV = psB.tile([HD, nmv, E], F32, tag="v")
                        for kk in range(NK):
                            for mv in range(nmv):
                                nc.tensor.matmul(
                                    psV[:, mv, :],
                                    xv[:, kk, si * nmv + mv, :],
                                    wv_sb[:, kk, :],
                                    start=(kk == 0), stop=False,
                                )
                        for mv in range(nmv):
                            # bias: ones^T @ bv_row as a final K=1 matmul
                            nc.tensor.matmul(
                                psV[:, mv, :], allones[0:1, :], bv_sb,
                                start=False, stop=True,
                            )
                        nc.scalar.activation(
                            v_sb[:, si * nmv:(si + 1) * nmv, :], psV,
                            AF.Identity)

            # ---------------- Phase C: attention ----------------
            with tc.tile_pool(name="outT", bufs=1) as outT_pool:
                outT_sb = outT_pool.tile([HD, NH, S], BF16)
                wo_sb = outT_pool.tile([HD, NH, D], BF16)
                # prefetch WO now; x/w pools above are closed
                nc.sync.dma_start(
                    out=wo_sb, in_=wot.rearrange("p (k d) -> p k d", d=D))
                with (
                    tc.tile_pool(name="attws", bufs=2) as ws_pool,
                    tc.tile_pool(name="es", bufs=4) as es_pool,
                    tc.tile_pool(name="psS", bufs=2, space="PSUM") as psS_pool,
                    tc.tile_pool(name="psO", bufs=2, space="PSUM") as psO_pool,
                    tc.tile_pool(name="psN", bufs=2, space="PSUM") as psN_pool,
                ):
                    for h in range(NH):
                        for ib in range(NSC):
                            i0 = ib * SC
                            psO = psO_pool.tile([HD, SC], F32, tag="o")
                            ps_bc = psN_pool.tile([HD, SC], F32, tag="bc")
                            es_t = [None] * NP

                            def emit_av(p, h=h, psO=psO, ps_bc=ps_bc,
                                        es_t=es_t):
                                es, esp = es_t[p]
                                for jj in range(2):
                                    j = 2 * p + jj
                                    nc.tensor.matmul(
                                        psO,
                                        v_sb[:, j, h * HD:(h + 1) * HD],
                                        es[:, jj, :],
                                        start=(j == 0), stop=(j == NJ - 1),
                                    )
                                # denominator: one bcast matmul per pair on
                                # the DVE-precomputed es0+es1
                                nc.tensor.matmul(
                                    ps_bc, allones, esp,
                                    start=(p == 0), stop=(p == NP - 1),
                                )

                            for p in range(NP):
                                psS = psS_pool.tile([HD, 2, SC], F32, tag="s")
                                for jj in range(2):
                                    j = 2 * p + jj
                                    nc.tensor.matmul(
                                        psS[:, jj, :],
                                        k_sb[:, h, j * HD:(j + 1) * HD],
                                        q_sb[:, h, i0:i0 + SC],
                                        start=True, stop=True,
                                    )
                                if p >= 2:
                                    emit_av(p - 2)
                                es = es_pool.tile([HD, 2, SC], BF16, tag="es",
                                                  name="es")
                                nc.scalar.activation(es, psS, AF.Exp,
                                                     scale=scale)
                                esp = es_pool.tile([HD, SC], BF16, tag="esp",
                                                   name="esp")
                                nc.vector.tensor_add(
                                    esp, es[:, 0, :], es[:, 1, :])
                                es_t[p] = (es, esp)
                            emit_av(NP - 2)
                            emit_av(NP - 1)
                            # normalize: approx 1/d (18-bit, ample for bf16)
                            rcp = ws_pool.tile([HD, SC], F32, tag="rcp")
                            nc.vector.reciprocal_approx_fast(
                                out=rcp, in_=ps_bc)
                            nc.vector.tensor_mul(
                                outT_sb[:, h, i0:i0 + SC], psO, rcp)

                # ---------------- Phase D: WO projection ----------------
                with (
                    tc.tile_pool(name="og", bufs=2) as og_pool,
                    tc.tile_pool(name="psW", bufs=2, space="PSUM") as psW_pool,
                ):
                    for it in range(NIT):
                        og = og_pool.tile([HD, D], BF16, tag="og")
                        psW = psW_pool.tile([HD, ND, SC], F32, tag="w")
                        for kk in range(NH):
                            for nn in range(ND):
                                nc.tensor.matmul(
                                    psW[:, nn, :],
                                    outT_sb[:, kk, it * HD:(it + 1) * HD],
                                    wo_sb[:, kk, nn * SC:(nn + 1) * SC],
                                    start=(kk == 0), stop=(kk == NH - 1),
                                )
                        nc.scalar.activation(og, psW, AF.Identity)
                        nc.sync.dma_start(
                            out=out[it * HD:(it + 1) * HD, :], in_=og)

    nc.compile()
    return nc


# ---------------------------------------------------------------------------
# Host-side sharding helpers
# ---------------------------------------------------------------------------

def _bf16(a):
    import ml_dtypes
    return np.asarray(a).astype(ml_dtypes.bfloat16)


def make_in_map(x_b, wq_e, bq_e, wk_e, bk_e, wv_e, bv_e, wo_e):
    """Per-core input dict. x_b [S, D]; w*_e [E, D] row slices; wo_e [D, E]
    column slice; b*_e [E]."""
    E = wq_e.shape[0]
    S, D = x_b.shape
    HD = 128
    NH = E // HD
    NK = D // HD

    def w_hmajor(w_e):  # [E, D] -> [HD, NH*NK*HD]: p=k-row, (h, k, col)
        # entry [p, h, k, c] = w_e[h*HD+c, k*HD+p]
        t = w_e.reshape(NH, HD, NK, HD)        # [h, c, k, p]
        t = t.transpose(3, 0, 2, 1)            # [p, h, k, c]
        return _bf16(t.reshape(HD, NH * NK * HD))

    def wrelayout(wT):  # [D, E'] -> [HD, NK*E'] with k-tile-major columns
        Ew = wT.shape[1]
        return _bf16(
            wT.reshape(NK, HD, Ew).transpose(1, 0, 2).reshape(HD, NK * Ew))

    xT = x_b.T  # [D, S]
    return {
        "xr": _bf16(xT.reshape(NK, HD, S).transpose(1, 0, 2)
                    .reshape(HD, NK * S)),
        "wqt": w_hmajor(wq_e),
        "wkt": w_hmajor(wk_e),
        "wvt": wrelayout(wv_e.T),
        "wot": _bf16(
            wo_e.T.reshape(NH, HD, D).transpose(1, 0, 2).reshape(HD, NH * D)),
        "bqc": np.ascontiguousarray(bq_e.reshape(NH, HD).T),
        "bkc": np.ascontiguousarray(bk_e.reshape(NH, HD).T),
        "bvr": _bf16(bv_e.reshape(1, E)),
        "ones2d": _bf16(np.ones((HD, HD), np.float32)),
    }


def core_reference(x_b, wq_e, bq_e, wk_e, bk_e, wv_e, bv_e, wo_e):
    """Numpy reference for one core's partial output."""
    HD = 128
    q = x_b @ wq_e.T + bq_e
    k = x_b @ wk_e.T + bk_e
    v = x_b @ wv_e.T + bv_e
    E = q.shape[1]
    outs = []
    for h in range(E // HD):
        qh = q[:, h * HD:(h + 1) * HD]
        kh = k[:, h * HD:(h + 1) * HD]
        vh = v[:, h * HD:(h + 1) * HD]
        s = (qh @ kh.T) / math.sqrt(HD)
        p = np.exp(s)
        outs.append((p @ vh) / p.sum(-1, keepdims=True))
    o = np.concatenate(outs, axis=1)
    return o @ wo_e.T


# ---------------------------------------------------------------------------
# Entry point: full-input kernel with internal 8-way sharding
# ---------------------------------------------------------------------------

import os as _os

_NC_CACHE = {}


def _get_module():
    if "nc" not in _NC_CACHE:
        _NC_CACHE["nc"] = build_module(S=2048, D=2048, E=512)
    return _NC_CACHE["nc"]


def kernel(x, wq, bq, wk, bk, wv, bv, wo, bo):
    """Full inputs -> full output. 8 cores = 2 (batch) x 4 (head-group)."""
    from concourse import bass_utils

    x = np.asarray(x, dtype=np.float32)
    wq, bq = np.asarray(wq, np.float32), np.asarray(bq, np.float32)
    wk, bk = np.asarray(wk, np.float32), np.asarray(bk, np.float32)
    wv, bv = np.asarray(wv, np.float32), np.asarray(bv, np.float32)
    wo, bo = np.asarray(wo, np.float32), np.asarray(bo, np.float32)

    E = 512
    nc = _get_module()
    in_maps = []
    for c in range(8):
        b, g = divmod(c, 4)
        e = slice(g * E, (g + 1) * E)
        in_maps.append(make_in_map(
            x[b], wq[e], bq[e], wk[e], bk[e], wv[e], bv[e], wo[:, e]))

    trace = bool(int(_os.environ.get("ATTN_TRACE", "0")))
    kw = {}
    if trace:
        tmpdir = _os.environ.get("ATTN_TRACE_DIR") or None
        kw = dict(trace=True, tmpdir=tmpdir, trace_cores=[0])
    res = bass_utils.run_bass_kernel_spmd(
        nc, in_maps, core_ids=list(range(8)), **kw)
    if trace:
        print(f"HW exec time: {res.exec_time_ns} ns")
        _NC_CACHE["last_results"] = res

    y = np.empty((2, 2048, 2048), np.float32)
    for b in range(2):
        acc = res.results[4 * b]["out"].astype(np.float32)
        for g in range(1, 4):
            acc += res.results[4 * b + g]["out"].astype(np.float32)
        y[b] = acc + bo
    return y


# revision 13
# speedup vs baseline: 1.5086x; 1.0177x over previous
import sys
sys.path.insert(0, '/opt/trn_rl_repo')

"""Multi-head attention TP kernel for TRN2 — per-core program builder.

Sharding: 8 cores = 2 (batch) x 4 (head groups of 4 heads = 512 dims).
Each core computes, for its batch b and head-dim slice e:
    q = x[b] @ wq[e,:].T + bq[e]      (stored transposed: qT [E, S])
    k = x[b] @ wk[e,:].T + bk[e]      (kT [E, S])
    v = x[b] @ wv[e,:].T + bv[e]      (v [S, E])
    per head h (dh=128): score tiles K-major, exp (no max-sub; scores
    bounded ~|3|), softmax denominator via all-ones broadcast matmul,
    AV accumulated unnormalized, normalized on eviction via
    rsqrt(denom) applied twice (avoids slow DVE reciprocal).
    partial_out = attn_out @ wo[:, e].T   ([S, D] bf16; host sums 8
    partials + bo in fp32)

v2 notes (vs baseline):
- DMA order: HWDGE ring is FIFO per issuing engine, so the first-needed
  weights go first (wq h0/h1 -> x -> wq h2/h3 -> wk -> wv; wo at phase-C
  start). Baseline queued all of x first and idled the PE ~39us.
- Phase A as separate Q/K passes, h-outer, 4-bank PSUM quads ping-ponged
  across h; stationary weight tile reused by 4 consecutive matmuls; one
  2048-wide activation per head amortizes ACT fixed cost.
- Phase C processes j in pairs: 2-bank psS tile, one 1024-wide exp.
- Normalization: rsqrt on ACT + two DVE multiplies.
- Output bf16 (halves output DMA).
"""

import math

import numpy as np

import concourse.bass as bass
import concourse.tile as tile
from concourse import bacc, mybir

F32 = mybir.dt.float32
BF16 = mybir.dt.bfloat16
AF = mybir.ActivationFunctionType


def build_module(
    S=2048,          # sequence per core (one batch)
    D=2048,          # model dim
    E=512,           # head dims per core (4 heads x 128)
    enable_asserts=False,
):
    HD = 128
    SC = 512
    NK = D // HD        # proj contraction tiles
    NH = E // HD        # heads per core
    NSC = S // SC       # s-chunks / i-blocks
    NJ = S // HD        # attention j tiles
    NP = NJ // 2        # attention j pairs
    ND = D // SC        # WO n-chunks
    NIT = S // HD       # WO i tiles
    scale = 1.0 / math.sqrt(HD)

    nc = bacc.Bacc(
        "TRN2",
        target_bir_lowering=False,
        debug=False,
        enable_asserts=enable_asserts,
        num_devices=8,
    )

    # host-side layouts (see make_in_map):
    #   xr  [HD, NK*S]       k-tile-major xT
    #   wqt [HD, NH*NK*HD]   h-major, then k-tile, then head-col
    xr = nc.dram_tensor("xr", [HD, NK * S], BF16, kind="ExternalInput").ap()
    wqt = nc.dram_tensor("wqt", [HD, NH * NK * HD], BF16,
                         kind="ExternalInput").ap()
    wkt = nc.dram_tensor("wkt", [HD, NH * NK * HD], BF16,
                         kind="ExternalInput").ap()
    wvt = nc.dram_tensor("wvt", [HD, NK * E], BF16, kind="ExternalInput").ap()
    wot = nc.dram_tensor("wot", [HD, NH * D], BF16, kind="ExternalInput").ap()
    bqc = nc.dram_tensor("bqc", [HD, NH], F32, kind="ExternalInput").ap()
    bkc = nc.dram_tensor("bkc", [HD, NH], F32, kind="ExternalInput").ap()
    ones2d = nc.dram_tensor("ones2d", [HD, HD], BF16,
                            kind="ExternalInput").ap()
    out = nc.dram_tensor("out", [S, D], BF16, kind="ExternalOutput").ap()

    with tile.TileContext(nc) as tc:
        with (
            tc.tile_pool(name="qkv", bufs=1) as qkv_pool,
            tc.tile_pool(name="consts", bufs=1) as consts,
        ):
            q_sb = qkv_pool.tile([HD, NH, S], BF16)
            k_sb = qkv_pool.tile([HD, NH, S], BF16)
            v_sb = qkv_pool.tile([HD, NJ, E], BF16)

            bq_sb = consts.tile([HD, NH], F32)
            bk_sb = consts.tile([HD, NH], F32)
            allones = consts.tile([HD, HD], BF16)    # bcast-sum stationary
            # ones first: the warm-up matmuls only need this tile
            nc.sync.dma_start(out=allones, in_=ones2d)

            # -------- Phases A/B: Q,K then V projections, x resident -------
            with (
                tc.tile_pool(name="xall", bufs=1) as x_pool,
                tc.tile_pool(name="wqk", bufs=1) as w_pool,
            ):
                xall = x_pool.tile([HD, NK, S], BF16)
                xv = xall.rearrange("p k (t h) -> p k t h", h=HD)
                wq_sb = w_pool.tile([HD, NH, NK, HD], BF16, tag="wq")
                wk_sb = w_pool.tile([HD, NH, NK, HD], BF16, tag="wk")
                wv_sb = w_pool.tile([HD, NK, E], BF16, tag="wv")

                wqr = wqt.rearrange("p (h k c) -> p h k c", h=NH, k=NK)
                wkr = wkt.rearrange("p (h k c) -> p h k c", h=NH, k=NK)

                # DMA issue order == arrival order (FIFO ring):
                # wq h0/h1 first so the very first matmuls can start, then
                # x (k-tile-major, consumed k-outer), then the rest.
                nc.sync.dma_start(out=wq_sb[:, 0:1], in_=wqr[:, 0:1])
                nc.sync.dma_start(out=wq_sb[:, 1:2], in_=wqr[:, 1:2])
                nc.sync.dma_start(out=bq_sb, in_=bqc)
                for g in range(NK // 2):
                    nc.sync.dma_start(
                        out=xall[:, 2 * g:2 * g + 2, :],
                        in_=xr[:, 2 * g * S:(2 * g + 2) * S].rearrange(
                            "p (k s) -> p k s", s=S),
                    )
                nc.sync.dma_start(out=wq_sb[:, 2:4], in_=wqr[:, 2:4])
                nc.sync.dma_start(out=bk_sb, in_=bkc)
                nc.sync.dma_start(out=wk_sb[:, 0:2], in_=wkr[:, 0:2])
                nc.sync.dma_start(out=wk_sb[:, 2:4], in_=wkr[:, 2:4])
                for g4 in range(NK // 4):
                    nc.sync.dma_start(
                        out=wv_sb[:, 4 * g4:4 * g4 + 4, :],
                        in_=wvt[:, 4 * g4 * E:(4 * g4 + 4) * E].rearrange(
                            "p (k e) -> p k e", e=E))

                # warm-up matmuls on the (tiny, early-arriving) ones tile:
                # keeps the PE HAM counter busy during the input DMA wait
                # so real matmuls start at full clock
                with tc.tile_pool(name="psWm", bufs=1, space="PSUM") as psWm:
                    warm = psWm.tile([HD, HD], F32)
                    for _ in range(40):
                        nc.tensor.matmul(warm, allones, allones,
                                         start=True, stop=True)

                # ---- Q pass, then K pass: h-outer, stationary reused x4 ---
                with tc.tile_pool(name="psA", bufs=2, space="PSUM") as psA:
                    for (w_sb, b_sb, dst) in ((wq_sb, bq_sb, q_sb),
                                              (wk_sb, bk_sb, k_sb)):
                        for h in range(NH):
                            quad = psA.tile([HD, NSC, SC], F32, tag="q")
                            for kk in range(NK):
                                for si in range(NSC):
                                    nc.tensor.matmul(
                                        quad[:, si, :],
                                        w_sb[:, h, kk, :],
                                        xall[:, kk, si * SC:(si + 1) * SC],
                                        start=(kk == 0), stop=(kk == NK - 1),
                                    )
                            nc.scalar.activation(
                                dst[:, h, :], quad, AF.Identity,
                                bias=b_sb[:, h:h + 1],
                            )

                # ---- V projection reuses resident x as stationary tiles ---
                with tc.tile_pool(name="psB", bufs=2, space="PSUM") as psB:
                    for si in range(NSC):
                        nmv = SC // HD
                        psV = psB.tile([HD, nmv, E], F32, tag="v")
                        # no V bias here: softmax weights sum to 1, so the
                        # bias passes through attention unchanged and the
                        # host adds wo @ bv to the output instead
                        for kk in range(NK):
                            for mv in range(nmv):
                                nc.tensor.matmul(
                                    psV[:, mv, :],
                                    xv[:, kk, si * nmv + mv, :],
                                    wv_sb[:, kk, :],
                                    start=(kk == 0), stop=(kk == NK - 1),
                                )
                        nc.scalar.activation(
                            v_sb[:, si * nmv:(si + 1) * nmv, :], psV,
                            AF.Identity)

            # ---------------- Phase C: attention ----------------
            with tc.tile_pool(name="outT", bufs=1) as outT_pool:
                outT_sb = outT_pool.tile([HD, NH, S], BF16)
                wo_sb = outT_pool.tile([HD, NH, D], BF16)
                # prefetch WO now; x/w pools above are closed
                nc.sync.dma_start(
                    out=wo_sb, in_=wot.rearrange("p (k d) -> p k d", d=D))
                with (
                    tc.tile_pool(name="attws", bufs=2) as ws_pool,
                    tc.tile_pool(name="es", bufs=4) as es_pool,
                    tc.tile_pool(name="psS", bufs=2, space="PSUM") as psS_pool,
                    tc.tile_pool(name="psO", bufs=2, space="PSUM") as psO_pool,
                    tc.tile_pool(name="psN", bufs=2, space="PSUM") as psN_pool,
                ):
                    for h in range(NH):
                        for ib in range(NSC):
                            i0 = ib * SC
                            psO = psO_pool.tile([HD, SC], F32, tag="o")
                            ps_bc = psN_pool.tile([HD, SC], F32, tag="bc")
                            es_t = [None] * NP

                            def emit_av(p, h=h, psO=psO, ps_bc=ps_bc,
                                        es_t=es_t):
                                es, esp = es_t[p]
                                for jj in range(2):
                                    j = 2 * p + jj
                                    nc.tensor.matmul(
                                        psO,
                                        v_sb[:, j, h * HD:(h + 1) * HD],
                                        es[:, jj, :],
                                        start=(j == 0), stop=(j == NJ - 1),
                                    )
                                # denominator: one bcast matmul per pair on
                                # the DVE-precomputed es0+es1
                                nc.tensor.matmul(
                                    ps_bc, allones, esp,
                                    start=(p == 0), stop=(p == NP - 1),
                                )

                            for p in range(NP):
                                psS = psS_pool.tile([HD, 2, SC], F32, tag="s")
                                for jj in range(2):
                                    j = 2 * p + jj
                                    nc.tensor.matmul(
                                        psS[:, jj, :],
                                        k_sb[:, h, j * HD:(j + 1) * HD],
                                        q_sb[:, h, i0:i0 + SC],
                                        start=True, stop=True,
                                    )
                                if p >= 2:
                                    emit_av(p - 2)
                                es = es_pool.tile([HD, 2, SC], BF16, tag="es",
                                                  name="es")
                                nc.scalar.activation(es, psS, AF.Exp,
                                                     scale=scale)
                                esp = es_pool.tile([HD, SC], BF16, tag="esp",
                                                   name="esp")
                                nc.vector.tensor_add(
                                    esp, es[:, 0, :], es[:, 1, :])
                                es_t[p] = (es, esp)
                            emit_av(NP - 2)
                            emit_av(NP - 1)
                            # normalize: approx 1/d (18-bit, ample for bf16)
                            rcp = ws_pool.tile([HD, SC], F32, tag="rcp")
                            nc.vector.reciprocal_approx_fast(
                                out=rcp, in_=ps_bc)
                            nc.vector.tensor_mul(
                                outT_sb[:, h, i0:i0 + SC], psO, rcp)

                # ---------------- Phase D: WO projection ----------------
                with (
                    tc.tile_pool(name="og", bufs=2) as og_pool,
                    tc.tile_pool(name="psW", bufs=2, space="PSUM") as psW_pool,
                ):
                    for it in range(NIT):
                        og = og_pool.tile([HD, D], BF16, tag="og")
                        psW = psW_pool.tile([HD, ND, SC], F32, tag="w")
                        for kk in range(NH):
                            for nn in range(ND):
                                nc.tensor.matmul(
                                    psW[:, nn, :],
                                    outT_sb[:, kk, it * HD:(it + 1) * HD],
                                    wo_sb[:, kk, nn * SC:(nn + 1) * SC],
                                    start=(kk == 0), stop=(kk == NH - 1),
                                )
                        if it < NIT - 1:
                            nc.scalar.activation(og, psW, AF.Identity)
                            nc.sync.dma_start(
                                out=out[it * HD:(it + 1) * HD, :], in_=og)
                        else:
                            # last tile: fine-grained eviction shortens the
                            # kernel tail (ACT+DMA per 512-chunk)
                            for nn in range(ND):
                                nc.scalar.activation(
                                    og[:, nn * SC:(nn + 1) * SC],
                                    psW[:, nn, :], AF.Identity)
                                nc.sync.dma_start(
                                    out=out[it * HD:(it + 1) * HD,
                                            nn * SC:(nn + 1) * SC],
                                    in_=og[:, nn * SC:(nn + 1) * SC])

    nc.compile()
    return nc


# ---------------------------------------------------------------------------
# Host-side sharding helpers
# ---------------------------------------------------------------------------

def _bf16(a):
    import ml_dtypes
    return np.asarray(a).astype(ml_dtypes.bfloat16)


def make_in_map(x_b, wq_e, bq_e, wk_e, bk_e, wv_e, bv_e, wo_e):
    """Per-core input dict. x_b [S, D]; w*_e [E, D] row slices; wo_e [D, E]
    column slice; b*_e [E]."""
    E = wq_e.shape[0]
    S, D = x_b.shape
    HD = 128
    NH = E // HD
    NK = D // HD

    def w_hmajor(w_e):  # [E, D] -> [HD, NH*NK*HD]: p=k-row, (h, k, col)
        # entry [p, h, k, c] = w_e[h*HD+c, k*HD+p]
        t = w_e.reshape(NH, HD, NK, HD)        # [h, c, k, p]
        t = t.transpose(3, 0, 2, 1)            # [p, h, k, c]
        return _bf16(t.reshape(HD, NH * NK * HD))

    def wrelayout(wT):  # [D, E'] -> [HD, NK*E'] with k-tile-major columns
        Ew = wT.shape[1]
        return _bf16(
            wT.reshape(NK, HD, Ew).transpose(1, 0, 2).reshape(HD, NK * Ew))

    xT = x_b.T  # [D, S]
    return {
        "xr": _bf16(xT.reshape(NK, HD, S).transpose(1, 0, 2)
                    .reshape(HD, NK * S)),
        "wqt": w_hmajor(wq_e),
        "wkt": w_hmajor(wk_e),
        "wvt": wrelayout(wv_e.T),
        "wot": _bf16(
            wo_e.T.reshape(NH, HD, D).transpose(1, 0, 2).reshape(HD, NH * D)),
        "bqc": np.ascontiguousarray(bq_e.reshape(NH, HD).T),
        "bkc": np.ascontiguousarray(bk_e.reshape(NH, HD).T),
        "ones2d": _bf16(np.ones((HD, HD), np.float32)),
    }


def core_reference(x_b, wq_e, bq_e, wk_e, bk_e, wv_e, bv_e, wo_e):
    """Numpy reference for one core's partial output."""
    HD = 128
    q = x_b @ wq_e.T + bq_e
    k = x_b @ wk_e.T + bk_e
    v = x_b @ wv_e.T + bv_e
    E = q.shape[1]
    outs = []
    for h in range(E // HD):
        qh = q[:, h * HD:(h + 1) * HD]
        kh = k[:, h * HD:(h + 1) * HD]
        vh = v[:, h * HD:(h + 1) * HD]
        s = (qh @ kh.T) / math.sqrt(HD)
        p = np.exp(s)
        outs.append((p @ vh) / p.sum(-1, keepdims=True))
    o = np.concatenate(outs, axis=1)
    return o @ wo_e.T


# ---------------------------------------------------------------------------
# Entry point: full-input kernel with internal 8-way sharding
# ---------------------------------------------------------------------------

import os as _os

_NC_CACHE = {}


def _get_module():
    if "nc" not in _NC_CACHE:
        _NC_CACHE["nc"] = build_module(S=2048, D=2048, E=512)
    return _NC_CACHE["nc"]


def kernel(x, wq, bq, wk, bk, wv, bv, wo, bo):
    """Full inputs -> full output. 8 cores = 2 (batch) x 4 (head-group)."""
    from concourse import bass_utils

    x = np.asarray(x, dtype=np.float32)
    wq, bq = np.asarray(wq, np.float32), np.asarray(bq, np.float32)
    wk, bk = np.asarray(wk, np.float32), np.asarray(bk, np.float32)
    wv, bv = np.asarray(wv, np.float32), np.asarray(bv, np.float32)
    wo, bo = np.asarray(wo, np.float32), np.asarray(bo, np.float32)

    E = 512
    nc = _get_module()
    in_maps = []
    for c in range(8):
        b, g = divmod(c, 4)
        e = slice(g * E, (g + 1) * E)
        in_maps.append(make_in_map(
            x[b], wq[e], bq[e], wk[e], bk[e], wv[e], bv[e], wo[:, e]))

    trace = bool(int(_os.environ.get("ATTN_TRACE", "0")))
    kw = {}
    if trace:
        tmpdir = _os.environ.get("ATTN_TRACE_DIR") or None
        kw = dict(trace=True, tmpdir=tmpdir, trace_cores=[0])
    res = bass_utils.run_bass_kernel_spmd(
        nc, in_maps, core_ids=list(range(8)), **kw)
    if trace:
        print(f"HW exec time: {res.exec_time_ns} ns")
        _NC_CACHE["last_results"] = res

    # V bias folded out of the device kernel: softmax rows sum to 1, so
    # its contribution to the output is the constant row wo @ bv (+ bo).
    const_row = bo + wo @ bv
    y = np.empty((2, 2048, 2048), np.float32)
    for b in range(2):
        acc = res.results[4 * b]["out"].astype(np.float32)
        for g in range(1, 4):
            acc += res.results[4 * b + g]["out"].astype(np.float32)
        y[b] = acc + const_row
    return y


# revision 16
# speedup vs baseline: 1.5190x; 1.0069x over previous
import sys
sys.path.insert(0, '/opt/trn_rl_repo')

"""Multi-head attention TP kernel for TRN2 — per-core program builder.

Sharding: 8 cores = 2 (batch) x 4 (head groups of 4 heads = 512 dims).
Each core computes, for its batch b and head-dim slice e:
    q = x[b] @ wq[e,:].T + bq[e]      (stored transposed: qT [E, S])
    k = x[b] @ wk[e,:].T + bk[e]      (kT [E, S])
    v = x[b] @ wv[e,:].T + bv[e]      (v [S, E])
    per head h (dh=128): score tiles K-major, exp (no max-sub; scores
    bounded ~|3|), softmax denominator via all-ones broadcast matmul,
    AV accumulated unnormalized, normalized on eviction via
    rsqrt(denom) applied twice (avoids slow DVE reciprocal).
    partial_out = attn_out @ wo[:, e].T   ([S, D] bf16; host sums 8
    partials + bo in fp32)

v2 notes (vs baseline):
- DMA order: HWDGE ring is FIFO per issuing engine, so the first-needed
  weights go first (wq h0/h1 -> x -> wq h2/h3 -> wk -> wv; wo at phase-C
  start). Baseline queued all of x first and idled the PE ~39us.
- Phase A as separate Q/K passes, h-outer, 4-bank PSUM quads ping-ponged
  across h; stationary weight tile reused by 4 consecutive matmuls; one
  2048-wide activation per head amortizes ACT fixed cost.
- Phase C processes j in pairs: 2-bank psS tile, one 1024-wide exp.
- Normalization: rsqrt on ACT + two DVE multiplies.
- Output bf16 (halves output DMA).
"""

import math

import numpy as np

import concourse.bass as bass
import concourse.tile as tile
from concourse import bacc, mybir

F32 = mybir.dt.float32
BF16 = mybir.dt.bfloat16
AF = mybir.ActivationFunctionType


def build_module(
    S=2048,          # sequence per core (one batch)
    D=2048,          # model dim
    E=512,           # head dims per core (4 heads x 128)
    enable_asserts=False,
):
    HD = 128
    SC = 512
    NK = D // HD        # proj contraction tiles
    NH = E // HD        # heads per core
    NSC = S // SC       # s-chunks / i-blocks
    NJ = S // HD        # attention j tiles
    NP = NJ // 2        # attention j pairs
    ND = D // SC        # WO n-chunks
    NIT = S // HD       # WO i tiles
    scale = 1.0 / math.sqrt(HD)

    nc = bacc.Bacc(
        "TRN2",
        target_bir_lowering=False,
        debug=False,
        enable_asserts=enable_asserts,
        num_devices=8,
    )

    # host-side layouts (see make_in_map):
    #   xr  [HD, NK*S]       k-tile-major xT
    #   wqt [HD, NH*NK*HD]   h-major, then k-tile, then head-col
    xr = nc.dram_tensor("xr", [HD, NK * S], BF16, kind="ExternalInput").ap()
    wqt = nc.dram_tensor("wqt", [HD, NH * NK * HD], BF16,
                         kind="ExternalInput").ap()
    wkt = nc.dram_tensor("wkt", [HD, NH * NK * HD], BF16,
                         kind="ExternalInput").ap()
    wvt = nc.dram_tensor("wvt", [HD, NK * E], BF16, kind="ExternalInput").ap()
    wot = nc.dram_tensor("wot", [HD, NH * D], BF16, kind="ExternalInput").ap()
    bqc = nc.dram_tensor("bqc", [HD, NH], F32, kind="ExternalInput").ap()
    bkc = nc.dram_tensor("bkc", [HD, NH], F32, kind="ExternalInput").ap()
    ones2d = nc.dram_tensor("ones2d", [HD, HD], BF16,
                            kind="ExternalInput").ap()
    out = nc.dram_tensor("out", [S, D], BF16, kind="ExternalOutput").ap()

    with tile.TileContext(nc) as tc:
        with (
            tc.tile_pool(name="qkv", bufs=1) as qkv_pool,
            tc.tile_pool(name="consts", bufs=1) as consts,
        ):
            q_sb = qkv_pool.tile([HD, NH, S], BF16)
            k_sb = qkv_pool.tile([HD, NH, S], BF16)
            v_sb = qkv_pool.tile([HD, NJ, E], BF16)

            bq_sb = consts.tile([HD, NH], F32)
            bk_sb = consts.tile([HD, NH], F32)
            allones = consts.tile([HD, HD], BF16)    # bcast-sum stationary
            # ones first: the warm-up matmuls only need this tile
            nc.sync.dma_start(out=allones, in_=ones2d)

            # -------- Phases A/B: Q,K then V projections, x resident -------
            with (
                tc.tile_pool(name="xall", bufs=1) as x_pool,
                tc.tile_pool(name="wqk", bufs=1) as w_pool,
            ):
                xall = x_pool.tile([HD, NK, S], BF16)
                xv = xall.rearrange("p k (t h) -> p k t h", h=HD)
                wq_sb = w_pool.tile([HD, NH, NK, HD], BF16, tag="wq")
                wk_sb = w_pool.tile([HD, NH, NK, HD], BF16, tag="wk")
                wv_sb = w_pool.tile([HD, NK, E], BF16, tag="wv")

                wqr = wqt.rearrange("p (h k c) -> p h k c", h=NH, k=NK)
                wkr = wkt.rearrange("p (h k c) -> p h k c", h=NH, k=NK)

                # DMA issue order == arrival order (FIFO ring):
                # wq h0/h1 first so the very first matmuls can start, then
                # x (k-tile-major, consumed k-outer), then the rest.
                nc.sync.dma_start(out=wq_sb[:, 0:1], in_=wqr[:, 0:1])
                nc.sync.dma_start(out=wq_sb[:, 1:2], in_=wqr[:, 1:2])
                nc.sync.dma_start(out=bq_sb, in_=bqc)
                for g in range(NK // 2):
                    nc.sync.dma_start(
                        out=xall[:, 2 * g:2 * g + 2, :],
                        in_=xr[:, 2 * g * S:(2 * g + 2) * S].rearrange(
                            "p (k s) -> p k s", s=S),
                    )
                nc.sync.dma_start(out=wq_sb[:, 2:4], in_=wqr[:, 2:4])
                nc.sync.dma_start(out=bk_sb, in_=bkc)
                nc.sync.dma_start(out=wk_sb[:, 0:2], in_=wkr[:, 0:2])
                nc.sync.dma_start(out=wk_sb[:, 2:4], in_=wkr[:, 2:4])
                for g4 in range(NK // 4):
                    nc.sync.dma_start(
                        out=wv_sb[:, 4 * g4:4 * g4 + 4, :],
                        in_=wvt[:, 4 * g4 * E:(4 * g4 + 4) * E].rearrange(
                            "p (k e) -> p k e", e=E))

                # warm-up matmuls on the (tiny, early-arriving) ones tile:
                # keeps the PE HAM counter busy during the input DMA wait
                # so real matmuls start at full clock
                with tc.tile_pool(name="psWm", bufs=1, space="PSUM") as psWm:
                    warm = psWm.tile([HD, HD], F32)
                    for _ in range(40):
                        nc.tensor.matmul(warm, allones, allones,
                                         start=True, stop=True)

                # ---- Q pass, then K pass: h-outer, stationary reused x4 ---
                with tc.tile_pool(name="psA", bufs=2, space="PSUM") as psA:
                    for (w_sb, b_sb, dst) in ((wq_sb, bq_sb, q_sb),
                                              (wk_sb, bk_sb, k_sb)):
                        for h in range(NH):
                            quad = psA.tile([HD, NSC, SC], F32, tag="q")
                            for kk in range(NK):
                                for si in range(NSC):
                                    nc.tensor.matmul(
                                        quad[:, si, :],
                                        w_sb[:, h, kk, :],
                                        xall[:, kk, si * SC:(si + 1) * SC],
                                        start=(kk == 0), stop=(kk == NK - 1),
                                    )
                            nc.scalar.activation(
                                dst[:, h, :], quad, AF.Identity,
                                bias=b_sb[:, h:h + 1],
                            )

                # ---- V projection reuses resident x as stationary tiles ---
                with tc.tile_pool(name="psB", bufs=2, space="PSUM") as psB:
                    for si in range(NSC):
                        nmv = SC // HD
                        psV = psB.tile([HD, nmv, E], F32, tag="v")
                        # no V bias here: softmax weights sum to 1, so the
                        # bias passes through attention unchanged and the
                        # host adds wo @ bv to the output instead
                        for kk in range(NK):
                            for mv in range(nmv):
                                nc.tensor.matmul(
                                    psV[:, mv, :],
                                    xv[:, kk, si * nmv + mv, :],
                                    wv_sb[:, kk, :],
                                    start=(kk == 0), stop=(kk == NK - 1),
                                )
                        nc.scalar.activation(
                            v_sb[:, si * nmv:(si + 1) * nmv, :], psV,
                            AF.Identity)

            # ---------------- Phase C: attention ----------------
            with tc.tile_pool(name="outT", bufs=1) as outT_pool:
                outT_sb = outT_pool.tile([HD, NH, S], BF16)
                wo_sb = outT_pool.tile([HD, NH, D], BF16)
                # prefetch WO now; x/w pools above are closed
                nc.sync.dma_start(
                    out=wo_sb, in_=wot.rearrange("p (k d) -> p k d", d=D))
                with (
                    tc.tile_pool(name="attws", bufs=2) as ws_pool,
                    tc.tile_pool(name="es", bufs=4) as es_pool,
                    tc.tile_pool(name="psS", bufs=2, space="PSUM") as psS_pool,
                    tc.tile_pool(name="psO", bufs=2, space="PSUM") as psO_pool,
                    tc.tile_pool(name="psN", bufs=2, space="PSUM") as psN_pool,
                ):
                    for h in range(NH):
                        for ib in range(NSC):
                            i0 = ib * SC
                            psO = psO_pool.tile([HD, SC], F32, tag="o")
                            ps_bc = psN_pool.tile([HD, SC], F32, tag="bc")
                            es_t = [None] * NP

                            def emit_av(p, h=h, psO=psO, ps_bc=ps_bc,
                                        es_t=es_t):
                                es, esq = es_t[p]
                                for jj in range(2):
                                    j = 2 * p + jj
                                    nc.tensor.matmul(
                                        psO,
                                        v_sb[:, j, h * HD:(h + 1) * HD],
                                        es[:, jj, :],
                                        start=(j == 0), stop=(j == NJ - 1),
                                    )
                                # denominator: one bcast matmul per QUAD of
                                # j tiles, on the DVE-summed exp tiles
                                if esq is not None:
                                    nc.tensor.matmul(
                                        ps_bc, allones, esq,
                                        start=(p == 1), stop=(p == NP - 1),
                                    )

                            esp_t = [None] * NP
                            for p in range(NP):
                                psS = psS_pool.tile([HD, 2, SC], F32, tag="s")
                                for jj in range(2):
                                    j = 2 * p + jj
                                    nc.tensor.matmul(
                                        psS[:, jj, :],
                                        k_sb[:, h, j * HD:(j + 1) * HD],
                                        q_sb[:, h, i0:i0 + SC],
                                        start=True, stop=True,
                                    )
                                if p >= 2:
                                    emit_av(p - 2)
                                es = es_pool.tile([HD, 2, SC], BF16, tag="es",
                                                  name="es")
                                nc.scalar.activation(es, psS, AF.Exp,
                                                     scale=scale)
                                esp = es_pool.tile([HD, SC], BF16, tag="esp",
                                                   name="esp")
                                nc.vector.tensor_add(
                                    esp, es[:, 0, :], es[:, 1, :])
                                esp_t[p] = esp
                                esq = None
                                if p % 2 == 1:
                                    esq = es_pool.tile([HD, SC], BF16,
                                                       tag="esq", name="esq")
                                    nc.vector.tensor_add(
                                        esq, esp_t[p - 1], esp)
                                es_t[p] = (es, esq)
                            emit_av(NP - 2)
                            emit_av(NP - 1)
                            # normalize: approx 1/d (18-bit, ample for bf16)
                            rcp = ws_pool.tile([HD, SC], F32, tag="rcp")
                            nc.vector.reciprocal_approx_fast(
                                out=rcp, in_=ps_bc)
                            nc.vector.tensor_mul(
                                outT_sb[:, h, i0:i0 + SC], psO, rcp)

                # ---------------- Phase D: WO projection ----------------
                with (
                    tc.tile_pool(name="og", bufs=2) as og_pool,
                    tc.tile_pool(name="psW", bufs=2, space="PSUM") as psW_pool,
                ):
                    for it in range(NIT):
                        og = og_pool.tile([HD, D], BF16, tag="og")
                        psW = psW_pool.tile([HD, ND, SC], F32, tag="w")
                        if it < NIT - 1:
                            for kk in range(NH):
                                for nn in range(ND):
                                    nc.tensor.matmul(
                                        psW[:, nn, :],
                                        outT_sb[:, kk, it * HD:(it + 1) * HD],
                                        wo_sb[:, kk, nn * SC:(nn + 1) * SC],
                                        start=(kk == 0), stop=(kk == NH - 1),
                                    )
                            nc.scalar.activation(og, psW, AF.Identity)
                            nc.sync.dma_start(
                                out=out[it * HD:(it + 1) * HD, :], in_=og)
                        else:
                            # last tile: nn-outer so each 512-chunk evicts
                            # (ACT+DMA) under the remaining matmuls,
                            # shortening the kernel tail
                            for nn in range(ND):
                                for kk in range(NH):
                                    nc.tensor.matmul(
                                        psW[:, nn, :],
                                        outT_sb[:, kk, it * HD:(it + 1) * HD],
                                        wo_sb[:, kk, nn * SC:(nn + 1) * SC],
                                        start=(kk == 0), stop=(kk == NH - 1),
                                    )
                                nc.scalar.activation(
                                    og[:, nn * SC:(nn + 1) * SC],
                                    psW[:, nn, :], AF.Identity)
                                nc.sync.dma_start(
                                    out=out[it * HD:(it + 1) * HD,
                                            nn * SC:(nn + 1) * SC],
                                    in_=og[:, nn * SC:(nn + 1) * SC])

    nc.compile()
    return nc


# ---------------------------------------------------------------------------
# Host-side sharding helpers
# ---------------------------------------------------------------------------

def _bf16(a):
    import ml_dtypes
    return np.asarray(a).astype(ml_dtypes.bfloat16)


def make_in_map(x_b, wq_e, bq_e, wk_e, bk_e, wv_e, bv_e, wo_e):
    """Per-core input dict. x_b [S, D]; w*_e [E, D] row slices; wo_e [D, E]
    column slice; b*_e [E]."""
    E = wq_e.shape[0]
    S, D = x_b.shape
    HD = 128
    NH = E // HD
    NK = D // HD

    def w_hmajor(w_e):  # [E, D] -> [HD, NH*NK*HD]: p=k-row, (h, k, col)
        # entry [p, h, k, c] = w_e[h*HD+c, k*HD+p]
        t = w_e.reshape(NH, HD, NK, HD)        # [h, c, k, p]
        t = t.transpose(3, 0, 2, 1)            # [p, h, k, c]
        return _bf16(t.reshape(HD, NH * NK * HD))

    def wrelayout(wT):  # [D, E'] -> [HD, NK*E'] with k-tile-major columns
        Ew = wT.shape[1]
        return _bf16(
            wT.reshape(NK, HD, Ew).transpose(1, 0, 2).reshape(HD, NK * Ew))

    xT = x_b.T  # [D, S]
    return {
        "xr": _bf16(xT.reshape(NK, HD, S).transpose(1, 0, 2)
                    .reshape(HD, NK * S)),
        "wqt": w_hmajor(wq_e),
        "wkt": w_hmajor(wk_e),
        "wvt": wrelayout(wv_e.T),
        "wot": _bf16(
            wo_e.T.reshape(NH, HD, D).transpose(1, 0, 2).reshape(HD, NH * D)),
        "bqc": np.ascontiguousarray(bq_e.reshape(NH, HD).T),
        "bkc": np.ascontiguousarray(bk_e.reshape(NH, HD).T),
        "ones2d": _bf16(np.ones((HD, HD), np.float32)),
    }


def core_reference(x_b, wq_e, bq_e, wk_e, bk_e, wv_e, bv_e, wo_e):
    """Numpy reference for one core's partial output."""
    HD = 128
    q = x_b @ wq_e.T + bq_e
    k = x_b @ wk_e.T + bk_e
    v = x_b @ wv_e.T + bv_e
    E = q.shape[1]
    outs = []
    for h in range(E // HD):
        qh = q[:, h * HD:(h + 1) * HD]
        kh = k[:, h * HD:(h + 1) * HD]
        vh = v[:, h * HD:(h + 1) * HD]
        s = (qh @ kh.T) / math.sqrt(HD)
        p = np.exp(s)
        outs.append((p @ vh) / p.sum(-1, keepdims=True))
    o = np.concatenate(outs, axis=1)
    return o @ wo_e.T


# ---------------------------------------------------------------------------
# Entry point: full-input kernel with internal 8-way sharding
# ---------------------------------------------------------------------------

import os as _os

_NC_CACHE = {}


def _get_module():
    if "nc" not in _NC_CACHE:
        _NC_CACHE["nc"] = build_module(S=2048, D=2048, E=512)
    return _NC_CACHE["nc"]


def kernel(x, wq, bq, wk, bk, wv, bv, wo, bo):
    """Full inputs -> full output. 8 cores = 2 (batch) x 4 (head-group)."""
    from concourse import bass_utils

    x = np.asarray(x, dtype=np.float32)
    wq, bq = np.asarray(wq, np.float32), np.asarray(bq, np.float32)
    wk, bk = np.asarray(wk, np.float32), np.asarray(bk, np.float32)
    wv, bv = np.asarray(wv, np.float32), np.asarray(bv, np.float32)
    wo, bo = np.asarray(wo, np.float32), np.asarray(bo, np.float32)

    E = 512
    nc = _get_module()
    in_maps = []
    for c in range(8):
        b, g = divmod(c, 4)
        e = slice(g * E, (g + 1) * E)
        in_maps.append(make_in_map(
            x[b], wq[e], bq[e], wk[e], bk[e], wv[e], bv[e], wo[:, e]))

    trace = bool(int(_os.environ.get("ATTN_TRACE", "0")))
    kw = {}
    if trace:
        tmpdir = _os.environ.get("ATTN_TRACE_DIR") or None
        kw = dict(trace=True, tmpdir=tmpdir, trace_cores=[0])
    res = bass_utils.run_bass_kernel_spmd(
        nc, in_maps, core_ids=list(range(8)), **kw)
    if trace:
        print(f"HW exec time: {res.exec_time_ns} ns")
        _NC_CACHE["last_results"] = res

    # V bias folded out of the device kernel: softmax rows sum to 1, so
    # its contribution to the output is the constant row wo @ bv (+ bo).
    const_row = bo + wo @ bv
    y = np.empty((2, 2048, 2048), np.float32)
    for b in range(2):
        acc = res.results[4 * b]["out"].astype(np.float32)
        for g in range(1, 4):
            acc += res.results[4 * b + g]["out"].astype(np.float32)
        y[b] = acc + const_row
    return y


# revision 18
# speedup vs baseline: 1.5452x; 1.0173x over previous
import sys
sys.path.insert(0, '/opt/trn_rl_repo')

"""Multi-head attention TP kernel for TRN2 — per-core program builder.

Sharding: 8 cores = 2 (batch) x 4 (head groups of 4 heads = 512 dims).
Each core computes, for its batch b and head-dim slice e:
    q = x[b] @ wq[e,:].T + bq[e]      (stored transposed: qT [E, S])
    k = x[b] @ wk[e,:].T + bk[e]      (kT [E, S])
    v = x[b] @ wv[e,:].T + bv[e]      (v [S, E])
    per head h (dh=128): score tiles K-major, exp (no max-sub; scores
    bounded ~|3|), softmax denominator via all-ones broadcast matmul,
    AV accumulated unnormalized, normalized on eviction via
    rsqrt(denom) applied twice (avoids slow DVE reciprocal).
    partial_out = attn_out @ wo[:, e].T   ([S, D] bf16; host sums 8
    partials + bo in fp32)

v2 notes (vs baseline):
- DMA order: HWDGE ring is FIFO per issuing engine, so the first-needed
  weights go first (wq h0/h1 -> x -> wq h2/h3 -> wk -> wv; wo at phase-C
  start). Baseline queued all of x first and idled the PE ~39us.
- Phase A as separate Q/K passes, h-outer, 4-bank PSUM quads ping-ponged
  across h; stationary weight tile reused by 4 consecutive matmuls; one
  2048-wide activation per head amortizes ACT fixed cost.
- Phase C processes j in pairs: 2-bank psS tile, one 1024-wide exp.
- Normalization: rsqrt on ACT + two DVE multiplies.
- Output bf16 (halves output DMA).
"""

import math

import numpy as np

import concourse.bass as bass
import concourse.tile as tile
from concourse import bacc, mybir

F32 = mybir.dt.float32
BF16 = mybir.dt.bfloat16
AF = mybir.ActivationFunctionType


def build_module(
    S=2048,          # sequence per core (one batch)
    D=2048,          # model dim
    E=512,           # head dims per core (4 heads x 128)
    enable_asserts=False,
):
    HD = 128
    SC = 512
    NK = D // HD        # proj contraction tiles
    NH = E // HD        # heads per core
    NSC = S // SC       # s-chunks / i-blocks
    NJ = S // HD        # attention j tiles
    NP = NJ // 2        # attention j pairs
    ND = D // SC        # WO n-chunks
    NIT = S // HD       # WO i tiles
    scale = 1.0 / math.sqrt(HD)

    nc = bacc.Bacc(
        "TRN2",
        target_bir_lowering=False,
        debug=False,
        enable_asserts=enable_asserts,
        num_devices=8,
    )

    # host-side layouts (see make_in_map):
    #   xr  [HD, NK*S]       k-tile-major xT
    #   wqt [HD, NH*NK*HD]   h-major, then k-tile, then head-col
    xr = nc.dram_tensor("xr", [HD, NK * S], BF16, kind="ExternalInput").ap()
    wqt = nc.dram_tensor("wqt", [HD, NH * NK * HD], BF16,
                         kind="ExternalInput").ap()
    wkt = nc.dram_tensor("wkt", [HD, NH * NK * HD], BF16,
                         kind="ExternalInput").ap()
    wvt = nc.dram_tensor("wvt", [HD, NK * E], BF16, kind="ExternalInput").ap()
    wot = nc.dram_tensor("wot", [HD, NH * D], BF16, kind="ExternalInput").ap()
    bqc = nc.dram_tensor("bqc", [HD, NH], F32, kind="ExternalInput").ap()
    bkc = nc.dram_tensor("bkc", [HD, NH], F32, kind="ExternalInput").ap()
    ones2d = nc.dram_tensor("ones2d", [HD, HD], BF16,
                            kind="ExternalInput").ap()
    out = nc.dram_tensor("out", [S, D], BF16, kind="ExternalOutput").ap()

    with tile.TileContext(nc) as tc:
        with (
            tc.tile_pool(name="qkv", bufs=1) as qkv_pool,
            tc.tile_pool(name="consts", bufs=1) as consts,
        ):
            q_sb = qkv_pool.tile([HD, NH, S], BF16)
            k_sb = qkv_pool.tile([HD, NH, S], BF16)
            v_sb = qkv_pool.tile([HD, NJ, E], BF16)

            bq_sb = consts.tile([HD, NH], F32)
            bk_sb = consts.tile([HD, NH], F32)
            allones = consts.tile([HD, HD], BF16)    # bcast-sum stationary
            # ones first: the warm-up matmuls only need this tile
            nc.sync.dma_start(out=allones, in_=ones2d)

            # -------- Phases A/B: Q,K then V projections, x resident -------
            with (
                tc.tile_pool(name="xall", bufs=1) as x_pool,
                tc.tile_pool(name="wqk", bufs=1) as w_pool,
            ):
                xall = x_pool.tile([HD, NK, S], BF16)
                xv = xall.rearrange("p k (t h) -> p k t h", h=HD)
                wq_sb = w_pool.tile([HD, NH, NK, HD], BF16, tag="wq")
                wk_sb = w_pool.tile([HD, NH, NK, HD], BF16, tag="wk")
                wv_sb = w_pool.tile([HD, NK, E], BF16, tag="wv")

                wqr = wqt.rearrange("p (h k c) -> p h k c", h=NH, k=NK)
                wkr = wkt.rearrange("p (h k c) -> p h k c", h=NH, k=NK)

                # DMA issue order == arrival order (FIFO ring):
                # wq h0/h1 first so the very first matmuls can start, then
                # x (k-tile-major, consumed k-outer), then the rest.
                nc.sync.dma_start(out=wq_sb[:, 0:1], in_=wqr[:, 0:1])
                nc.sync.dma_start(out=wq_sb[:, 1:2], in_=wqr[:, 1:2])
                nc.sync.dma_start(out=bq_sb, in_=bqc)
                for g in range(NK // 2):
                    nc.sync.dma_start(
                        out=xall[:, 2 * g:2 * g + 2, :],
                        in_=xr[:, 2 * g * S:(2 * g + 2) * S].rearrange(
                            "p (k s) -> p k s", s=S),
                    )
                nc.sync.dma_start(out=wq_sb[:, 2:4], in_=wqr[:, 2:4])
                nc.sync.dma_start(out=bk_sb, in_=bkc)
                nc.sync.dma_start(out=wk_sb[:, 0:2], in_=wkr[:, 0:2])
                nc.sync.dma_start(out=wk_sb[:, 2:4], in_=wkr[:, 2:4])
                for g4 in range(NK // 4):
                    nc.sync.dma_start(
                        out=wv_sb[:, 4 * g4:4 * g4 + 4, :],
                        in_=wvt[:, 4 * g4 * E:(4 * g4 + 4) * E].rearrange(
                            "p (k e) -> p k e", e=E))

                # warm-up matmuls on the (tiny, early-arriving) ones tile:
                # keeps the PE HAM counter busy during the input DMA wait
                # so real matmuls start at full clock
                with tc.tile_pool(name="psWm", bufs=1, space="PSUM") as psWm:
                    warm = psWm.tile([HD, HD], F32)
                    for _ in range(40):
                        nc.tensor.matmul(warm, allones, allones,
                                         start=True, stop=True)

                # ---- Q pass, then K pass: h-outer, stationary reused x4 ---
                with tc.tile_pool(name="psA", bufs=2, space="PSUM") as psA:
                    for (w_sb, b_sb, dst) in ((wq_sb, bq_sb, q_sb),
                                              (wk_sb, bk_sb, k_sb)):
                        for h in range(NH):
                            quad = psA.tile([HD, NSC, SC], F32, tag="q")
                            for kk in range(NK):
                                for si in range(NSC):
                                    nc.tensor.matmul(
                                        quad[:, si, :],
                                        w_sb[:, h, kk, :],
                                        xall[:, kk, si * SC:(si + 1) * SC],
                                        start=(kk == 0), stop=(kk == NK - 1),
                                    )
                            nc.scalar.activation(
                                dst[:, h, :], quad, AF.Identity,
                                bias=b_sb[:, h:h + 1],
                            )

                    # -- V projection: same pool/tag as A's quads, so the
                    # pool keeps rotating with no A->B drain barrier --
                    for si in range(NSC):
                        nmv = SC // HD
                        psV = psA.tile([HD, nmv, E], F32, tag="q")
                        # no V bias here: softmax weights sum to 1, so the
                        # bias passes through attention unchanged and the
                        # host adds wo @ bv to the output instead
                        for kk in range(NK):
                            for mv in range(nmv):
                                nc.tensor.matmul(
                                    psV[:, mv, :],
                                    xv[:, kk, si * nmv + mv, :],
                                    wv_sb[:, kk, :],
                                    start=(kk == 0), stop=(kk == NK - 1),
                                )
                        nc.scalar.activation(
                            v_sb[:, si * nmv:(si + 1) * nmv, :], psV,
                            AF.Identity)

            # ---------------- Phase C: attention ----------------
            with tc.tile_pool(name="outT", bufs=1) as outT_pool:
                outT_sb = outT_pool.tile([HD, NH, S], BF16)
                wo_sb = outT_pool.tile([HD, NH, D], BF16)
                # prefetch WO now; x/w pools above are closed
                nc.sync.dma_start(
                    out=wo_sb, in_=wot.rearrange("p (k d) -> p k d", d=D))
                with (
                    tc.tile_pool(name="attws", bufs=2) as ws_pool,
                    tc.tile_pool(name="es", bufs=4) as es_pool,
                    tc.tile_pool(name="psS", bufs=2, space="PSUM") as psS_pool,
                    tc.tile_pool(name="psO", bufs=2, space="PSUM") as psO_pool,
                    tc.tile_pool(name="psN", bufs=2, space="PSUM") as psN_pool,
                ):
                    for h in range(NH):
                        for ib in range(NSC):
                            i0 = ib * SC
                            psO = psO_pool.tile([HD, SC], F32, tag="o")
                            ps_bc = psN_pool.tile([HD, SC], F32, tag="bc")
                            es_t = [None] * NP

                            def emit_av(p, h=h, psO=psO, ps_bc=ps_bc,
                                        es_t=es_t):
                                es, esq = es_t[p]
                                for jj in range(2):
                                    j = 2 * p + jj
                                    nc.tensor.matmul(
                                        psO,
                                        v_sb[:, j, h * HD:(h + 1) * HD],
                                        es[:, jj, :],
                                        start=(j == 0), stop=(j == NJ - 1),
                                    )
                                # denominator: one bcast matmul per QUAD of
                                # j tiles, on the DVE-summed exp tiles
                                if esq is not None:
                                    nc.tensor.matmul(
                                        ps_bc, allones, esq,
                                        start=(p == 1), stop=(p == NP - 1),
                                    )

                            esp_t = [None] * NP
                            for p in range(NP):
                                psS = psS_pool.tile([HD, 2, SC], F32, tag="s")
                                for jj in range(2):
                                    j = 2 * p + jj
                                    nc.tensor.matmul(
                                        psS[:, jj, :],
                                        k_sb[:, h, j * HD:(j + 1) * HD],
                                        q_sb[:, h, i0:i0 + SC],
                                        start=True, stop=True,
                                    )
                                if p >= 2:
                                    emit_av(p - 2)
                                es = es_pool.tile([HD, 2, SC], BF16, tag="es",
                                                  name="es")
                                nc.scalar.activation(es, psS, AF.Exp,
                                                     scale=scale)
                                esp = es_pool.tile([HD, SC], BF16, tag="esp",
                                                   name="esp")
                                nc.vector.tensor_add(
                                    esp, es[:, 0, :], es[:, 1, :])
                                esp_t[p] = esp
                                esq = None
                                if p % 2 == 1:
                                    esq = es_pool.tile([HD, SC], BF16,
                                                       tag="esq", name="esq")
                                    nc.vector.tensor_add(
                                        esq, esp_t[p - 1], esp)
                                es_t[p] = (es, esq)
                            emit_av(NP - 2)
                            emit_av(NP - 1)
                            # normalize: approx 1/d (18-bit, ample for bf16)
                            rcp = ws_pool.tile([HD, SC], F32, tag="rcp")
                            nc.vector.reciprocal_approx_fast(
                                out=rcp, in_=ps_bc)
                            nc.vector.tensor_mul(
                                outT_sb[:, h, i0:i0 + SC], psO, rcp)

                # ---------------- Phase D: WO projection ----------------
                with (
                    tc.tile_pool(name="og", bufs=2) as og_pool,
                    tc.tile_pool(name="psW", bufs=2, space="PSUM") as psW_pool,
                ):
                    for it in range(NIT):
                        og = og_pool.tile([HD, D], BF16, tag="og")
                        psW = psW_pool.tile([HD, ND, SC], F32, tag="w")
                        for kk in range(NH):
                            for nn in range(ND):
                                nc.tensor.matmul(
                                    psW[:, nn, :],
                                    outT_sb[:, kk, it * HD:(it + 1) * HD],
                                    wo_sb[:, kk, nn * SC:(nn + 1) * SC],
                                    start=(kk == 0), stop=(kk == NH - 1),
                                )
                        if it < NIT - 1:
                            nc.scalar.activation(og, psW, AF.Identity)
                            nc.sync.dma_start(
                                out=out[it * HD:(it + 1) * HD, :], in_=og)
                        else:
                            # last tile: fine-grained eviction shortens the
                            # kernel tail (ACT+DMA per 512-chunk)
                            for nn in range(ND):
                                nc.scalar.activation(
                                    og[:, nn * SC:(nn + 1) * SC],
                                    psW[:, nn, :], AF.Identity)
                                nc.sync.dma_start(
                                    out=out[it * HD:(it + 1) * HD,
                                            nn * SC:(nn + 1) * SC],
                                    in_=og[:, nn * SC:(nn + 1) * SC])

    nc.compile()
    return nc


# ---------------------------------------------------------------------------
# Host-side sharding helpers
# ---------------------------------------------------------------------------

def _bf16(a):
    import ml_dtypes
    return np.asarray(a).astype(ml_dtypes.bfloat16)


def make_in_map(x_b, wq_e, bq_e, wk_e, bk_e, wv_e, bv_e, wo_e):
    """Per-core input dict. x_b [S, D]; w*_e [E, D] row slices; wo_e [D, E]
    column slice; b*_e [E]."""
    E = wq_e.shape[0]
    S, D = x_b.shape
    HD = 128
    NH = E // HD
    NK = D // HD

    def w_hmajor(w_e):  # [E, D] -> [HD, NH*NK*HD]: p=k-row, (h, k, col)
        # entry [p, h, k, c] = w_e[h*HD+c, k*HD+p]
        t = w_e.reshape(NH, HD, NK, HD)        # [h, c, k, p]
        t = t.transpose(3, 0, 2, 1)            # [p, h, k, c]
        return _bf16(t.reshape(HD, NH * NK * HD))

    def wrelayout(wT):  # [D, E'] -> [HD, NK*E'] with k-tile-major columns
        Ew = wT.shape[1]
        return _bf16(
            wT.reshape(NK, HD, Ew).transpose(1, 0, 2).reshape(HD, NK * Ew))

    xT = x_b.T  # [D, S]
    return {
        "xr": _bf16(xT.reshape(NK, HD, S).transpose(1, 0, 2)
                    .reshape(HD, NK * S)),
        "wqt": w_hmajor(wq_e),
        "wkt": w_hmajor(wk_e),
        "wvt": wrelayout(wv_e.T),
        "wot": _bf16(
            wo_e.T.reshape(NH, HD, D).transpose(1, 0, 2).reshape(HD, NH * D)),
        "bqc": np.ascontiguousarray(bq_e.reshape(NH, HD).T),
        "bkc": np.ascontiguousarray(bk_e.reshape(NH, HD).T),
        "ones2d": _bf16(np.ones((HD, HD), np.float32)),
    }


def core_reference(x_b, wq_e, bq_e, wk_e, bk_e, wv_e, bv_e, wo_e):
    """Numpy reference for one core's partial output."""
    HD = 128
    q = x_b @ wq_e.T + bq_e
    k = x_b @ wk_e.T + bk_e
    v = x_b @ wv_e.T + bv_e
    E = q.shape[1]
    outs = []
    for h in range(E // HD):
        qh = q[:, h * HD:(h + 1) * HD]
        kh = k[:, h * HD:(h + 1) * HD]
        vh = v[:, h * HD:(h + 1) * HD]
        s = (qh @ kh.T) / math.sqrt(HD)
        p = np.exp(s)
        outs.append((p @ vh) / p.sum(-1, keepdims=True))
    o = np.concatenate(outs, axis=1)
    return o @ wo_e.T


# ---------------------------------------------------------------------------
# Entry point: full-input kernel with internal 8-way sharding
# ---------------------------------------------------------------------------

import os as _os

_NC_CACHE = {}


def _get_module():
    if "nc" not in _NC_CACHE:
        _NC_CACHE["nc"] = build_module(S=2048, D=2048, E=512)
    return _NC_CACHE["nc"]


def kernel(x, wq, bq, wk, bk, wv, bv, wo, bo):
    """Full inputs -> full output. 8 cores = 2 (batch) x 4 (head-group)."""
    from concourse import bass_utils

    x = np.asarray(x, dtype=np.float32)
    wq, bq = np.asarray(wq, np.float32), np.asarray(bq, np.float32)
    wk, bk = np.asarray(wk, np.float32), np.asarray(bk, np.float32)
    wv, bv = np.asarray(wv, np.float32), np.asarray(bv, np.float32)
    wo, bo = np.asarray(wo, np.float32), np.asarray(bo, np.float32)

    E = 512
    nc = _get_module()
    in_maps = []
    for c in range(8):
        b, g = divmod(c, 4)
        e = slice(g * E, (g + 1) * E)
        in_maps.append(make_in_map(
            x[b], wq[e], bq[e], wk[e], bk[e], wv[e], bv[e], wo[:, e]))

    trace = bool(int(_os.environ.get("ATTN_TRACE", "0")))
    kw = {}
    if trace:
        tmpdir = _os.environ.get("ATTN_TRACE_DIR") or None
        kw = dict(trace=True, tmpdir=tmpdir, trace_cores=[0])
    res = bass_utils.run_bass_kernel_spmd(
        nc, in_maps, core_ids=list(range(8)), **kw)
    if trace:
        print(f"HW exec time: {res.exec_time_ns} ns")
        _NC_CACHE["last_results"] = res

    # V bias folded out of the device kernel: softmax rows sum to 1, so
    # its contribution to the output is the constant row wo @ bv (+ bo).
    const_row = bo + wo @ bv
    y = np.empty((2, 2048, 2048), np.float32)
    for b in range(2):
        acc = res.results[4 * b]["out"].astype(np.float32)
        for g in range(1, 4):
            acc += res.results[4 * b + g]["out"].astype(np.float32)
        y[b] = acc + const_row
    return y


# revision 20
# speedup vs baseline: 1.5518x; 1.0043x over previous
import sys
sys.path.insert(0, '/opt/trn_rl_repo')

"""Multi-head attention TP kernel for TRN2 — per-core program builder.

Sharding: 8 cores = 2 (batch) x 4 (head groups of 4 heads = 512 dims).
Each core computes, for its batch b and head-dim slice e:
    q = x[b] @ wq[e,:].T + bq[e]      (stored transposed: qT [E, S])
    k = x[b] @ wk[e,:].T + bk[e]      (kT [E, S])
    v = x[b] @ wv[e,:].T + bv[e]      (v [S, E])
    per head h (dh=128): score tiles K-major, exp (no max-sub; scores
    bounded ~|3|), softmax denominator via all-ones broadcast matmul,
    AV accumulated unnormalized, normalized on eviction via
    rsqrt(denom) applied twice (avoids slow DVE reciprocal).
    partial_out = attn_out @ wo[:, e].T   ([S, D] bf16; host sums 8
    partials + bo in fp32)

v2 notes (vs baseline):
- DMA order: HWDGE ring is FIFO per issuing engine, so the first-needed
  weights go first (wq h0/h1 -> x -> wq h2/h3 -> wk -> wv; wo at phase-C
  start). Baseline queued all of x first and idled the PE ~39us.
- Phase A as separate Q/K passes, h-outer, 4-bank PSUM quads ping-ponged
  across h; stationary weight tile reused by 4 consecutive matmuls; one
  2048-wide activation per head amortizes ACT fixed cost.
- Phase C processes j in pairs: 2-bank psS tile, one 1024-wide exp.
- Normalization: rsqrt on ACT + two DVE multiplies.
- Output bf16 (halves output DMA).
"""

import math

import numpy as np

import concourse.bass as bass
import concourse.tile as tile
from concourse import bacc, mybir

F32 = mybir.dt.float32
BF16 = mybir.dt.bfloat16
AF = mybir.ActivationFunctionType


def build_module(
    S=2048,          # sequence per core (one batch)
    D=2048,          # model dim
    E=512,           # head dims per core (4 heads x 128)
    enable_asserts=False,
):
    HD = 128
    SC = 512
    NK = D // HD        # proj contraction tiles
    NH = E // HD        # heads per core
    NSC = S // SC       # s-chunks / i-blocks
    NJ = S // HD        # attention j tiles
    NP = NJ // 2        # attention j pairs
    ND = D // SC        # WO n-chunks
    NIT = S // HD       # WO i tiles
    scale = 1.0 / math.sqrt(HD)

    nc = bacc.Bacc(
        "TRN2",
        target_bir_lowering=False,
        debug=False,
        enable_asserts=enable_asserts,
        num_devices=8,
    )

    # host-side layouts (see make_in_map):
    #   xr  [HD, NK*S]       k-tile-major xT
    #   wqt [HD, NH*NK*HD]   h-major, then k-tile, then head-col
    xr = nc.dram_tensor("xr", [HD, NK * S], BF16, kind="ExternalInput").ap()
    wqt = nc.dram_tensor("wqt", [HD, NH * NK * HD], BF16,
                         kind="ExternalInput").ap()
    wkt = nc.dram_tensor("wkt", [HD, NH * NK * HD], BF16,
                         kind="ExternalInput").ap()
    wvt = nc.dram_tensor("wvt", [HD, NK * E], BF16, kind="ExternalInput").ap()
    wot = nc.dram_tensor("wot", [HD, NH * D], BF16, kind="ExternalInput").ap()
    bqc = nc.dram_tensor("bqc", [HD, NH], F32, kind="ExternalInput").ap()
    bkc = nc.dram_tensor("bkc", [HD, NH], F32, kind="ExternalInput").ap()
    ones2d = nc.dram_tensor("ones2d", [HD, HD], BF16,
                            kind="ExternalInput").ap()
    out = nc.dram_tensor("out", [S, D], BF16, kind="ExternalOutput").ap()

    with tile.TileContext(nc) as tc:
        with (
            tc.tile_pool(name="qkv", bufs=1) as qkv_pool,
            tc.tile_pool(name="consts", bufs=1) as consts,
        ):
            q_sb = qkv_pool.tile([HD, NH, S], BF16)
            k_sb = qkv_pool.tile([HD, NH, S], BF16)
            v_sb = qkv_pool.tile([HD, NJ, E], BF16)

            bq_sb = consts.tile([HD, NH], F32)
            bk_sb = consts.tile([HD, NH], F32)
            allones = consts.tile([HD, HD], BF16)    # bcast-sum stationary
            # ones first: the warm-up matmuls only need this tile
            nc.sync.dma_start(out=allones, in_=ones2d)

            # -------- Phases A/B: Q,K then V projections, x resident -------
            with (
                tc.tile_pool(name="xall", bufs=1) as x_pool,
                tc.tile_pool(name="wqk", bufs=1) as w_pool,
            ):
                xall = x_pool.tile([HD, NK, S], BF16)
                xv = xall.rearrange("p k (t h) -> p k t h", h=HD)
                wq_sb = w_pool.tile([HD, NH, NK, HD], BF16, tag="wq")
                wk_sb = w_pool.tile([HD, NH, NK, HD], BF16, tag="wk")
                wv_sb = w_pool.tile([HD, NK, E], BF16, tag="wv")

                wqr = wqt.rearrange("p (h k c) -> p h k c", h=NH, k=NK)
                wkr = wkt.rearrange("p (h k c) -> p h k c", h=NH, k=NK)

                # DMA issue order == arrival order (FIFO ring):
                # wq h0/h1 first so the very first matmuls can start, then
                # x (k-tile-major, consumed k-outer), then the rest.
                nc.sync.dma_start(out=wq_sb[:, 0:1], in_=wqr[:, 0:1])
                nc.sync.dma_start(out=wq_sb[:, 1:2], in_=wqr[:, 1:2])
                nc.sync.dma_start(out=bq_sb, in_=bqc)
                for g in range(NK // 2):
                    nc.sync.dma_start(
                        out=xall[:, 2 * g:2 * g + 2, :],
                        in_=xr[:, 2 * g * S:(2 * g + 2) * S].rearrange(
                            "p (k s) -> p k s", s=S),
                    )
                nc.sync.dma_start(out=wq_sb[:, 2:4], in_=wqr[:, 2:4])
                nc.sync.dma_start(out=bk_sb, in_=bkc)
                nc.sync.dma_start(out=wk_sb[:, 0:2], in_=wkr[:, 0:2])
                nc.sync.dma_start(out=wk_sb[:, 2:4], in_=wkr[:, 2:4])
                for g4 in range(NK // 4):
                    nc.sync.dma_start(
                        out=wv_sb[:, 4 * g4:4 * g4 + 4, :],
                        in_=wvt[:, 4 * g4 * E:(4 * g4 + 4) * E].rearrange(
                            "p (k e) -> p k e", e=E))

                # warm-up matmuls on the (tiny, early-arriving) ones tile:
                # keeps the PE HAM counter busy during the input DMA wait
                # so real matmuls start at full clock
                with tc.tile_pool(name="psWm", bufs=1, space="PSUM") as psWm:
                    warm = psWm.tile([HD, HD], F32)
                    for _ in range(72):
                        nc.tensor.matmul(warm, allones, allones,
                                         start=True, stop=True)

                # ---- Q pass, then K pass: h-outer, stationary reused x4 ---
                with tc.tile_pool(name="psA", bufs=2, space="PSUM") as psA:
                    for (w_sb, b_sb, dst) in ((wq_sb, bq_sb, q_sb),
                                              (wk_sb, bk_sb, k_sb)):
                        for h in range(NH):
                            quad = psA.tile([HD, NSC, SC], F32, tag="q")
                            for kk in range(NK):
                                for si in range(NSC):
                                    nc.tensor.matmul(
                                        quad[:, si, :],
                                        w_sb[:, h, kk, :],
                                        xall[:, kk, si * SC:(si + 1) * SC],
                                        start=(kk == 0), stop=(kk == NK - 1),
                                    )
                            nc.scalar.activation(
                                dst[:, h, :], quad, AF.Identity,
                                bias=b_sb[:, h:h + 1],
                            )

                    # -- V projection: same pool/tag as A's quads, so the
                    # pool keeps rotating with no A->B drain barrier --
                    for si in range(NSC):
                        nmv = SC // HD
                        psV = psA.tile([HD, nmv, E], F32, tag="q")
                        # no V bias here: softmax weights sum to 1, so the
                        # bias passes through attention unchanged and the
                        # host adds wo @ bv to the output instead
                        for kk in range(NK):
                            for mv in range(nmv):
                                nc.tensor.matmul(
                                    psV[:, mv, :],
                                    xv[:, kk, si * nmv + mv, :],
                                    wv_sb[:, kk, :],
                                    start=(kk == 0), stop=(kk == NK - 1),
                                )
                        nc.scalar.activation(
                            v_sb[:, si * nmv:(si + 1) * nmv, :], psV,
                            AF.Identity)

            # ---------------- Phase C: attention ----------------
            with tc.tile_pool(name="outT", bufs=1) as outT_pool:
                outT_sb = outT_pool.tile([HD, NH, S], BF16)
                wo_sb = outT_pool.tile([HD, NH, D], BF16)
                # prefetch WO now; x/w pools above are closed
                nc.sync.dma_start(
                    out=wo_sb, in_=wot.rearrange("p (k d) -> p k d", d=D))
                with (
                    tc.tile_pool(name="attws", bufs=2) as ws_pool,
                    tc.tile_pool(name="es", bufs=4) as es_pool,
                    tc.tile_pool(name="psS", bufs=2, space="PSUM") as psS_pool,
                    tc.tile_pool(name="psO", bufs=2, space="PSUM") as psO_pool,
                    tc.tile_pool(name="psN", bufs=2, space="PSUM") as psN_pool,
                ):
                    for h in range(NH):
                        for ib in range(NSC):
                            i0 = ib * SC
                            psO = psO_pool.tile([HD, SC], F32, tag="o")
                            ps_bc = psN_pool.tile([HD, SC], F32, tag="bc")
                            es_t = [None] * NP

                            def emit_av(p, h=h, psO=psO, ps_bc=ps_bc,
                                        es_t=es_t):
                                es, esq = es_t[p]
                                for jj in range(2):
                                    j = 2 * p + jj
                                    nc.tensor.matmul(
                                        psO,
                                        v_sb[:, j, h * HD:(h + 1) * HD],
                                        es[:, jj, :],
                                        start=(j == 0), stop=(j == NJ - 1),
                                    )
                                # denominator: one bcast matmul per QUAD of
                                # j tiles, on the DVE-summed exp tiles
                                if esq is not None:
                                    nc.tensor.matmul(
                                        ps_bc, allones, esq,
                                        start=(p == 1), stop=(p == NP - 1),
                                    )

                            esp_t = [None] * NP
                            for p in range(NP):
                                psS = psS_pool.tile([HD, 2, SC], F32, tag="s")
                                for jj in range(2):
                                    j = 2 * p + jj
                                    nc.tensor.matmul(
                                        psS[:, jj, :],
                                        k_sb[:, h, j * HD:(j + 1) * HD],
                                        q_sb[:, h, i0:i0 + SC],
                                        start=True, stop=True,
                                    )
                                if p >= 2:
                                    emit_av(p - 2)
                                es = es_pool.tile([HD, 2, SC], BF16, tag="es",
                                                  name="es")
                                nc.scalar.activation(es, psS, AF.Exp,
                                                     scale=scale)
                                esp = es_pool.tile([HD, SC], BF16, tag="esp",
                                                   name="esp")
                                nc.vector.tensor_add(
                                    esp, es[:, 0, :], es[:, 1, :])
                                esp_t[p] = esp
                                esq = None
                                if p % 2 == 1:
                                    esq = es_pool.tile([HD, SC], BF16,
                                                       tag="esq", name="esq")
                                    nc.vector.tensor_add(
                                        esq, esp_t[p - 1], esp)
                                es_t[p] = (es, esq)
                            emit_av(NP - 2)
                            emit_av(NP - 1)
                            # normalize: approx 1/d (18-bit, ample for bf16)
                            rcp = ws_pool.tile([HD, SC], F32, tag="rcp")
                            nc.vector.reciprocal_approx_fast(
                                out=rcp, in_=ps_bc)
                            nc.vector.tensor_mul(
                                outT_sb[:, h, i0:i0 + SC], psO, rcp)

                # ---------------- Phase D: WO projection ----------------
                with (
                    tc.tile_pool(name="og", bufs=2) as og_pool,
                    tc.tile_pool(name="psW", bufs=2, space="PSUM") as psW_pool,
                ):
                    for it in range(NIT):
                        og = og_pool.tile([HD, D], BF16, tag="og")
                        psW = psW_pool.tile([HD, ND, SC], F32, tag="w")
                        for kk in range(NH):
                            for nn in range(ND):
                                nc.tensor.matmul(
                                    psW[:, nn, :],
                                    outT_sb[:, kk, it * HD:(it + 1) * HD],
                                    wo_sb[:, kk, nn * SC:(nn + 1) * SC],
                                    start=(kk == 0), stop=(kk == NH - 1),
                                )
                        if it < NIT - 1:
                            nc.scalar.activation(og, psW, AF.Identity)
                            nc.sync.dma_start(
                                out=out[it * HD:(it + 1) * HD, :], in_=og)
                        else:
                            # last tile: fine-grained eviction shortens the
                            # kernel tail; chunks alternate ACT/DVE so the
                            # copies run on both engines concurrently
                            for nn in range(ND):
                                dst = og[:, nn * SC:(nn + 1) * SC]
                                if nn % 2 == 0:
                                    nc.scalar.activation(
                                        dst, psW[:, nn, :], AF.Identity)
                                else:
                                    nc.vector.tensor_copy(
                                        dst, psW[:, nn, :])
                                nc.sync.dma_start(
                                    out=out[it * HD:(it + 1) * HD,
                                            nn * SC:(nn + 1) * SC],
                                    in_=dst)

    nc.compile()
    return nc


# ---------------------------------------------------------------------------
# Host-side sharding helpers
# ---------------------------------------------------------------------------

def _bf16(a):
    import ml_dtypes
    return np.asarray(a).astype(ml_dtypes.bfloat16)


def make_in_map(x_b, wq_e, bq_e, wk_e, bk_e, wv_e, bv_e, wo_e):
    """Per-core input dict. x_b [S, D]; w*_e [E, D] row slices; wo_e [D, E]
    column slice; b*_e [E]."""
    E = wq_e.shape[0]
    S, D = x_b.shape
    HD = 128
    NH = E // HD
    NK = D // HD

    def w_hmajor(w_e):  # [E, D] -> [HD, NH*NK*HD]: p=k-row, (h, k, col)
        # entry [p, h, k, c] = w_e[h*HD+c, k*HD+p]
        t = w_e.reshape(NH, HD, NK, HD)        # [h, c, k, p]
        t = t.transpose(3, 0, 2, 1)            # [p, h, k, c]
        return _bf16(t.reshape(HD, NH * NK * HD))

    def wrelayout(wT):  # [D, E'] -> [HD, NK*E'] with k-tile-major columns
        Ew = wT.shape[1]
        return _bf16(
            wT.reshape(NK, HD, Ew).transpose(1, 0, 2).reshape(HD, NK * Ew))

    xT = x_b.T  # [D, S]
    return {
        "xr": _bf16(xT.reshape(NK, HD, S).transpose(1, 0, 2)
                    .reshape(HD, NK * S)),
        "wqt": w_hmajor(wq_e),
        "wkt": w_hmajor(wk_e),
        "wvt": wrelayout(wv_e.T),
        "wot": _bf16(
            wo_e.T.reshape(NH, HD, D).transpose(1, 0, 2).reshape(HD, NH * D)),
        "bqc": np.ascontiguousarray(bq_e.reshape(NH, HD).T),
        "bkc": np.ascontiguousarray(bk_e.reshape(NH, HD).T),
        "ones2d": _bf16(np.ones((HD, HD), np.float32)),
    }


def core_reference(x_b, wq_e, bq_e, wk_e, bk_e, wv_e, bv_e, wo_e):
    """Numpy reference for one core's partial output."""
    HD = 128
    q = x_b @ wq_e.T + bq_e
    k = x_b @ wk_e.T + bk_e
    v = x_b @ wv_e.T + bv_e
    E = q.shape[1]
    outs = []
    for h in range(E // HD):
        qh = q[:, h * HD:(h + 1) * HD]
        kh = k[:, h * HD:(h + 1) * HD]
        vh = v[:, h * HD:(h + 1) * HD]
        s = (qh @ kh.T) / math.sqrt(HD)
        p = np.exp(s)
        outs.append((p @ vh) / p.sum(-1, keepdims=True))
    o = np.concatenate(outs, axis=1)
    return o @ wo_e.T


# ---------------------------------------------------------------------------
# Entry point: full-input kernel with internal 8-way sharding
# ---------------------------------------------------------------------------

import os as _os

_NC_CACHE = {}


def _get_module():
    if "nc" not in _NC_CACHE:
        _NC_CACHE["nc"] = build_module(S=2048, D=2048, E=512)
    return _NC_CACHE["nc"]


def kernel(x, wq, bq, wk, bk, wv, bv, wo, bo):
    """Full inputs -> full output. 8 cores = 2 (batch) x 4 (head-group)."""
    from concourse import bass_utils

    x = np.asarray(x, dtype=np.float32)
    wq, bq = np.asarray(wq, np.float32), np.asarray(bq, np.float32)
    wk, bk = np.asarray(wk, np.float32), np.asarray(bk, np.float32)
    wv, bv = np.asarray(wv, np.float32), np.asarray(bv, np.float32)
    wo, bo = np.asarray(wo, np.float32), np.asarray(bo, np.float32)

    E = 512
    nc = _get_module()
    in_maps = []
    for c in range(8):
        b, g = divmod(c, 4)
        e = slice(g * E, (g + 1) * E)
        in_maps.append(make_in_map(
            x[b], wq[e], bq[e], wk[e], bk[e], wv[e], bv[e], wo[:, e]))

    trace = bool(int(_os.environ.get("ATTN_TRACE", "0")))
    kw = {}
    if trace:
        tmpdir = _os.environ.get("ATTN_TRACE_DIR") or None
        kw = dict(trace=True, tmpdir=tmpdir, trace_cores=[0])
    res = bass_utils.run_bass_kernel_spmd(
        nc, in_maps, core_ids=list(range(8)), **kw)
    if trace:
        print(f"HW exec time: {res.exec_time_ns} ns")
        _NC_CACHE["last_results"] = res

    # V bias folded out of the device kernel: softmax rows sum to 1, so
    # its contribution to the output is the constant row wo @ bv (+ bo).
    const_row = bo + wo @ bv
    y = np.empty((2, 2048, 2048), np.float32)
    for b in range(2):
        acc = res.results[4 * b]["out"].astype(np.float32)
        for g in range(1, 4):
            acc += res.results[4 * b + g]["out"].astype(np.float32)
        y[b] = acc + const_row
    return y


# revision 21
# speedup vs baseline: 1.5542x; 1.0016x over previous
import sys
sys.path.insert(0, '/opt/trn_rl_repo')

"""Multi-head attention TP kernel for TRN2 — per-core program builder.

Sharding: 8 cores = 2 (batch) x 4 (head groups of 4 heads = 512 dims).
Each core computes, for its batch b and head-dim slice e:
    q = x[b] @ wq[e,:].T + bq[e]      (stored transposed: qT [E, S])
    k = x[b] @ wk[e,:].T + bk[e]      (kT [E, S])
    v = x[b] @ wv[e,:].T + bv[e]      (v [S, E])
    per head h (dh=128): score tiles K-major, exp (no max-sub; scores
    bounded ~|3|), softmax denominator via all-ones broadcast matmul,
    AV accumulated unnormalized, normalized on eviction via
    rsqrt(denom) applied twice (avoids slow DVE reciprocal).
    partial_out = attn_out @ wo[:, e].T   ([S, D] bf16; host sums 8
    partials + bo in fp32)

v2 notes (vs baseline):
- DMA order: HWDGE ring is FIFO per issuing engine, so the first-needed
  weights go first (wq h0/h1 -> x -> wq h2/h3 -> wk -> wv; wo at phase-C
  start). Baseline queued all of x first and idled the PE ~39us.
- Phase A as separate Q/K passes, h-outer, 4-bank PSUM quads ping-ponged
  across h; stationary weight tile reused by 4 consecutive matmuls; one
  2048-wide activation per head amortizes ACT fixed cost.
- Phase C processes j in pairs: 2-bank psS tile, one 1024-wide exp.
- Normalization: rsqrt on ACT + two DVE multiplies.
- Output bf16 (halves output DMA).
"""

import math

import numpy as np

import concourse.bass as bass
import concourse.tile as tile
from concourse import bacc, mybir

F32 = mybir.dt.float32
BF16 = mybir.dt.bfloat16
AF = mybir.ActivationFunctionType


def build_module(
    S=2048,          # sequence per core (one batch)
    D=2048,          # model dim
    E=512,           # head dims per core (4 heads x 128)
    enable_asserts=False,
):
    HD = 128
    SC = 512
    NK = D // HD        # proj contraction tiles
    NH = E // HD        # heads per core
    NSC = S // SC       # s-chunks / i-blocks
    NJ = S // HD        # attention j tiles
    NP = NJ // 2        # attention j pairs
    ND = D // SC        # WO n-chunks
    NIT = S // HD       # WO i tiles
    scale = 1.0 / math.sqrt(HD)

    nc = bacc.Bacc(
        "TRN2",
        target_bir_lowering=False,
        debug=False,
        enable_asserts=enable_asserts,
        num_devices=8,
    )

    # host-side layouts (see make_in_map):
    #   xr  [HD, NK*S]       k-tile-major xT
    #   wqt [HD, NH*NK*HD]   h-major, then k-tile, then head-col
    xr = nc.dram_tensor("xr", [HD, NK * S], BF16, kind="ExternalInput").ap()
    wqt = nc.dram_tensor("wqt", [HD, NH * NK * HD], BF16,
                         kind="ExternalInput").ap()
    wkt = nc.dram_tensor("wkt", [HD, NH * NK * HD], BF16,
                         kind="ExternalInput").ap()
    wvt = nc.dram_tensor("wvt", [HD, NK * E], BF16, kind="ExternalInput").ap()
    wot = nc.dram_tensor("wot", [HD, NH * D], BF16, kind="ExternalInput").ap()
    bqc = nc.dram_tensor("bqc", [HD, NH], F32, kind="ExternalInput").ap()
    bkc = nc.dram_tensor("bkc", [HD, NH], F32, kind="ExternalInput").ap()
    ones2d = nc.dram_tensor("ones2d", [HD, HD], BF16,
                            kind="ExternalInput").ap()
    out = nc.dram_tensor("out", [S, D], BF16, kind="ExternalOutput").ap()

    with tile.TileContext(nc) as tc:
        with (
            tc.tile_pool(name="qkv", bufs=1) as qkv_pool,
            tc.tile_pool(name="consts", bufs=1) as consts,
        ):
            q_sb = qkv_pool.tile([HD, NH, S], BF16)
            k_sb = qkv_pool.tile([HD, NH, S], BF16)
            v_sb = qkv_pool.tile([HD, NJ, E], BF16)

            bq_sb = consts.tile([HD, NH], F32)
            bk_sb = consts.tile([HD, NH], F32)
            allones = consts.tile([HD, HD], BF16)    # bcast-sum stationary
            # ones first: the warm-up matmuls only need this tile
            nc.sync.dma_start(out=allones, in_=ones2d)

            # -------- Phases A/B: Q,K then V projections, x resident -------
            with (
                tc.tile_pool(name="xall", bufs=1) as x_pool,
                tc.tile_pool(name="wqk", bufs=1) as w_pool,
            ):
                xall = x_pool.tile([HD, NK, S], BF16)
                xv = xall.rearrange("p k (t h) -> p k t h", h=HD)
                wq_sb = w_pool.tile([HD, NH, NK, HD], BF16, tag="wq")
                wk_sb = w_pool.tile([HD, NH, NK, HD], BF16, tag="wk")
                wv_sb = w_pool.tile([HD, NK, E], BF16, tag="wv")

                wqr = wqt.rearrange("p (h k c) -> p h k c", h=NH, k=NK)
                wkr = wkt.rearrange("p (h k c) -> p h k c", h=NH, k=NK)

                # DMA issue order == arrival order (FIFO ring): smallest
                # first-needed pieces first so the first matmuls start
                # ~13us (ring latency floor) instead of waiting bulk x.
                nc.sync.dma_start(out=wq_sb[:, 0:1, 0:1], in_=wqr[:, 0:1, 0:1])
                nc.sync.dma_start(out=xall[:, 0:1, :],
                                  in_=xr[:, 0:S].rearrange(
                                      "p (k s) -> p k s", s=S))
                nc.sync.dma_start(out=wq_sb[:, 0:1, 1:NK],
                                  in_=wqr[:, 0:1, 1:NK])
                nc.sync.dma_start(out=xall[:, 1:2, :],
                                  in_=xr[:, S:2 * S].rearrange(
                                      "p (k s) -> p k s", s=S))
                nc.sync.dma_start(out=wq_sb[:, 1:2], in_=wqr[:, 1:2])
                nc.sync.dma_start(out=bq_sb, in_=bqc)
                for g in range(1, NK // 2):
                    nc.sync.dma_start(
                        out=xall[:, 2 * g:2 * g + 2, :],
                        in_=xr[:, 2 * g * S:(2 * g + 2) * S].rearrange(
                            "p (k s) -> p k s", s=S),
                    )
                nc.sync.dma_start(out=wq_sb[:, 2:4], in_=wqr[:, 2:4])
                nc.sync.dma_start(out=bk_sb, in_=bkc)
                nc.sync.dma_start(out=wk_sb[:, 0:2], in_=wkr[:, 0:2])
                nc.sync.dma_start(out=wk_sb[:, 2:4], in_=wkr[:, 2:4])
                for g4 in range(NK // 4):
                    nc.sync.dma_start(
                        out=wv_sb[:, 4 * g4:4 * g4 + 4, :],
                        in_=wvt[:, 4 * g4 * E:(4 * g4 + 4) * E].rearrange(
                            "p (k e) -> p k e", e=E))

                # warm-up matmuls on the (tiny, early-arriving) ones tile:
                # keeps the PE HAM counter busy during the input DMA wait
                # so real matmuls start at full clock
                with tc.tile_pool(name="psWm", bufs=1, space="PSUM") as psWm:
                    warm = psWm.tile([HD, HD], F32)
                    for _ in range(72):
                        nc.tensor.matmul(warm, allones, allones,
                                         start=True, stop=True)

                # ---- Q pass, then K pass: h-outer, stationary reused x4 ---
                with tc.tile_pool(name="psA", bufs=2, space="PSUM") as psA:
                    for (w_sb, b_sb, dst) in ((wq_sb, bq_sb, q_sb),
                                              (wk_sb, bk_sb, k_sb)):
                        for h in range(NH):
                            quad = psA.tile([HD, NSC, SC], F32, tag="q")
                            for kk in range(NK):
                                for si in range(NSC):
                                    nc.tensor.matmul(
                                        quad[:, si, :],
                                        w_sb[:, h, kk, :],
                                        xall[:, kk, si * SC:(si + 1) * SC],
                                        start=(kk == 0), stop=(kk == NK - 1),
                                    )
                            nc.scalar.activation(
                                dst[:, h, :], quad, AF.Identity,
                                bias=b_sb[:, h:h + 1],
                            )

                    # -- V projection: same pool/tag as A's quads, so the
                    # pool keeps rotating with no A->B drain barrier --
                    for si in range(NSC):
                        nmv = SC // HD
                        psV = psA.tile([HD, nmv, E], F32, tag="q")
                        # no V bias here: softmax weights sum to 1, so the
                        # bias passes through attention unchanged and the
                        # host adds wo @ bv to the output instead
                        for kk in range(NK):
                            for mv in range(nmv):
                                nc.tensor.matmul(
                                    psV[:, mv, :],
                                    xv[:, kk, si * nmv + mv, :],
                                    wv_sb[:, kk, :],
                                    start=(kk == 0), stop=(kk == NK - 1),
                                )
                        nc.scalar.activation(
                            v_sb[:, si * nmv:(si + 1) * nmv, :], psV,
                            AF.Identity)

            # ---------------- Phase C: attention ----------------
            with tc.tile_pool(name="outT", bufs=1) as outT_pool:
                outT_sb = outT_pool.tile([HD, NH, S], BF16)
                wo_sb = outT_pool.tile([HD, NH, D], BF16)
                # prefetch WO now; x/w pools above are closed
                nc.sync.dma_start(
                    out=wo_sb, in_=wot.rearrange("p (k d) -> p k d", d=D))
                with (
                    tc.tile_pool(name="attws", bufs=2) as ws_pool,
                    tc.tile_pool(name="es", bufs=4) as es_pool,
                    tc.tile_pool(name="psS", bufs=2, space="PSUM") as psS_pool,
                    tc.tile_pool(name="psO", bufs=2, space="PSUM") as psO_pool,
                    tc.tile_pool(name="psN", bufs=2, space="PSUM") as psN_pool,
                ):
                    for h in range(NH):
                        for ib in range(NSC):
                            i0 = ib * SC
                            psO = psO_pool.tile([HD, SC], F32, tag="o")
                            ps_bc = psN_pool.tile([HD, SC], F32, tag="bc")
                            es_t = [None] * NP

                            def emit_av(p, h=h, psO=psO, ps_bc=ps_bc,
                                        es_t=es_t):
                                es, esq = es_t[p]
                                for jj in range(2):
                                    j = 2 * p + jj
                                    nc.tensor.matmul(
                                        psO,
                                        v_sb[:, j, h * HD:(h + 1) * HD],
                                        es[:, jj, :],
                                        start=(j == 0), stop=(j == NJ - 1),
                                    )
                                # denominator: one bcast matmul per QUAD of
                                # j tiles, on the DVE-summed exp tiles
                                if esq is not None:
                                    nc.tensor.matmul(
                                        ps_bc, allones, esq,
                                        start=(p == 1), stop=(p == NP - 1),
                                    )

                            esp_t = [None] * NP
                            for p in range(NP):
                                psS = psS_pool.tile([HD, 2, SC], F32, tag="s")
                                for jj in range(2):
                                    j = 2 * p + jj
                                    nc.tensor.matmul(
                                        psS[:, jj, :],
                                        k_sb[:, h, j * HD:(j + 1) * HD],
                                        q_sb[:, h, i0:i0 + SC],
                                        start=True, stop=True,
                                    )
                                if p >= 2:
                                    emit_av(p - 2)
                                es = es_pool.tile([HD, 2, SC], BF16, tag="es",
                                                  name="es")
                                nc.scalar.activation(es, psS, AF.Exp,
                                                     scale=scale)
                                esp = es_pool.tile([HD, SC], BF16, tag="esp",
                                                   name="esp")
                                nc.vector.tensor_add(
                                    esp, es[:, 0, :], es[:, 1, :])
                                esp_t[p] = esp
                                esq = None
                                if p % 2 == 1:
                                    esq = es_pool.tile([HD, SC], BF16,
                                                       tag="esq", name="esq")
                                    nc.vector.tensor_add(
                                        esq, esp_t[p - 1], esp)
                                es_t[p] = (es, esq)
                            emit_av(NP - 2)
                            emit_av(NP - 1)
                            # normalize: approx 1/d (18-bit, ample for bf16)
                            rcp = ws_pool.tile([HD, SC], F32, tag="rcp")
                            nc.vector.reciprocal_approx_fast(
                                out=rcp, in_=ps_bc)
                            nc.vector.tensor_mul(
                                outT_sb[:, h, i0:i0 + SC], psO, rcp)

                # ---------------- Phase D: WO projection ----------------
                with (
                    tc.tile_pool(name="og", bufs=2) as og_pool,
                    tc.tile_pool(name="psW", bufs=2, space="PSUM") as psW_pool,
                ):
                    for it in range(NIT):
                        og = og_pool.tile([HD, D], BF16, tag="og")
                        psW = psW_pool.tile([HD, ND, SC], F32, tag="w")
                        for kk in range(NH):
                            for nn in range(ND):
                                nc.tensor.matmul(
                                    psW[:, nn, :],
                                    outT_sb[:, kk, it * HD:(it + 1) * HD],
                                    wo_sb[:, kk, nn * SC:(nn + 1) * SC],
                                    start=(kk == 0), stop=(kk == NH - 1),
                                )
                        if it < NIT - 1:
                            nc.scalar.activation(og, psW, AF.Identity)
                            nc.sync.dma_start(
                                out=out[it * HD:(it + 1) * HD, :], in_=og)
                        else:
                            # last tile: fine-grained eviction shortens the
                            # kernel tail; chunks alternate ACT/DVE so the
                            # copies run on both engines concurrently
                            for nn in range(ND):
                                dst = og[:, nn * SC:(nn + 1) * SC]
                                if nn % 2 == 0:
                                    nc.scalar.activation(
                                        dst, psW[:, nn, :], AF.Identity)
                                else:
                                    nc.vector.tensor_copy(
                                        dst, psW[:, nn, :])
                                nc.sync.dma_start(
                                    out=out[it * HD:(it + 1) * HD,
                                            nn * SC:(nn + 1) * SC],
                                    in_=dst)

    nc.compile()
    return nc


# ---------------------------------------------------------------------------
# Host-side sharding helpers
# ---------------------------------------------------------------------------

def _bf16(a):
    import ml_dtypes
    return np.asarray(a).astype(ml_dtypes.bfloat16)


def make_in_map(x_b, wq_e, bq_e, wk_e, bk_e, wv_e, bv_e, wo_e):
    """Per-core input dict. x_b [S, D]; w*_e [E, D] row slices; wo_e [D, E]
    column slice; b*_e [E]."""
    E = wq_e.shape[0]
    S, D = x_b.shape
    HD = 128
    NH = E // HD
    NK = D // HD

    def w_hmajor(w_e):  # [E, D] -> [HD, NH*NK*HD]: p=k-row, (h, k, col)
        # entry [p, h, k, c] = w_e[h*HD+c, k*HD+p]
        t = w_e.reshape(NH, HD, NK, HD)        # [h, c, k, p]
        t = t.transpose(3, 0, 2, 1)            # [p, h, k, c]
        return _bf16(t.reshape(HD, NH * NK * HD))

    def wrelayout(wT):  # [D, E'] -> [HD, NK*E'] with k-tile-major columns
        Ew = wT.shape[1]
        return _bf16(
            wT.reshape(NK, HD, Ew).transpose(1, 0, 2).reshape(HD, NK * Ew))

    xT = x_b.T  # [D, S]
    return {
        "xr": _bf16(xT.reshape(NK, HD, S).transpose(1, 0, 2)
                    .reshape(HD, NK * S)),
        "wqt": w_hmajor(wq_e),
        "wkt": w_hmajor(wk_e),
        "wvt": wrelayout(wv_e.T),
        "wot": _bf16(
            wo_e.T.reshape(NH, HD, D).transpose(1, 0, 2).reshape(HD, NH * D)),
        "bqc": np.ascontiguousarray(bq_e.reshape(NH, HD).T),
        "bkc": np.ascontiguousarray(bk_e.reshape(NH, HD).T),
        "ones2d": _bf16(np.ones((HD, HD), np.float32)),
    }


def core_reference(x_b, wq_e, bq_e, wk_e, bk_e, wv_e, bv_e, wo_e):
    """Numpy reference for one core's partial output."""
    HD = 128
    q = x_b @ wq_e.T + bq_e
    k = x_b @ wk_e.T + bk_e
    v = x_b @ wv_e.T + bv_e
    E = q.shape[1]
    outs = []
    for h in range(E // HD):
        qh = q[:, h * HD:(h + 1) * HD]
        kh = k[:, h * HD:(h + 1) * HD]
        vh = v[:, h * HD:(h + 1) * HD]
        s = (qh @ kh.T) / math.sqrt(HD)
        p = np.exp(s)
        outs.append((p @ vh) / p.sum(-1, keepdims=True))
    o = np.concatenate(outs, axis=1)
    return o @ wo_e.T


# ---------------------------------------------------------------------------
# Entry point: full-input kernel with internal 8-way sharding
# ---------------------------------------------------------------------------

import os as _os

_NC_CACHE = {}


def _get_module():
    if "nc" not in _NC_CACHE:
        _NC_CACHE["nc"] = build_module(S=2048, D=2048, E=512)
    return _NC_CACHE["nc"]


def kernel(x, wq, bq, wk, bk, wv, bv, wo, bo):
    """Full inputs -> full output. 8 cores = 2 (batch) x 4 (head-group)."""
    from concourse import bass_utils

    x = np.asarray(x, dtype=np.float32)
    wq, bq = np.asarray(wq, np.float32), np.asarray(bq, np.float32)
    wk, bk = np.asarray(wk, np.float32), np.asarray(bk, np.float32)
    wv, bv = np.asarray(wv, np.float32), np.asarray(bv, np.float32)
    wo, bo = np.asarray(wo, np.float32), np.asarray(bo, np.float32)

    E = 512
    nc = _get_module()
    in_maps = []
    for c in range(8):
        b, g = divmod(c, 4)
        e = slice(g * E, (g + 1) * E)
        in_maps.append(make_in_map(
            x[b], wq[e], bq[e], wk[e], bk[e], wv[e], bv[e], wo[:, e]))

    trace = bool(int(_os.environ.get("ATTN_TRACE", "0")))
    kw = {}
    if trace:
        tmpdir = _os.environ.get("ATTN_TRACE_DIR") or None
        kw = dict(trace=True, tmpdir=tmpdir, trace_cores=[0])
    res = bass_utils.run_bass_kernel_spmd(
        nc, in_maps, core_ids=list(range(8)), **kw)
    if trace:
        print(f"HW exec time: {res.exec_time_ns} ns")
        _NC_CACHE["last_results"] = res

    # V bias folded out of the device kernel: softmax rows sum to 1, so
    # its contribution to the output is the constant row wo @ bv (+ bo).
    const_row = bo + wo @ bv
    y = np.empty((2, 2048, 2048), np.float32)
    for b in range(2):
        acc = res.results[4 * b]["out"].astype(np.float32)
        for g in range(1, 4):
            acc += res.results[4 * b + g]["out"].astype(np.float32)
        y[b] = acc + const_row
    return y
